# revision 11
# baseline (speedup 1.0000x reference)
"""Trainium2 Bass kernel: DiscriminatorRNN (GRU encode + autoregressive GRU decode).

Math (per reference):
  frames = concat(encoder_inputs, decoder_output) transposed to [T,B,I], T=74
  encode: h = GRUCell(frames[t], h)  for t in 0..73   (h0 = 0)
  decode: x0 = frames[0]; for d in 0..73: h = GRUCell(x_d, h); out_d = x_d + h@fc1_w.T + fc1_b; x_{d+1} = out_d
  logit[b] = sum_{d,i} out_d[b,i] * fc2_w[0, d*69+i] + fc2_b;  result = sigmoid(logit)  -> [B,1]

Distribution: pure data-parallel over batch, B=1024 -> 128 rows per core on 8 cores,
weights replicated, zero communication.

Per-core layout (one NeuronCore):
  - batch rows live on the matmul *stationary* operand (M=128), the GRU weights are
    the *moving* operand (float32r at full PE rate for free-dim >= 256).
  - gates g = x@Wih^T + h@Whh^T + biases accumulate in PSUM as [B=128, 3H] in six
    512-wide bursts (r0 r1 | z0 z1 | n0 n1), i_n and h_n kept in separate banks.
  - x is augmented with a constant 1.0 row; Wih^T gets a bias row, so biases ride the
    x matmul. h_n's b_hh bias comes from a K=1 ones-outer-product matmul.
  - the recurrent state is kept both as h [B,H] (for the elementwise update) and as
    h^T chunks [H,B] (the matmul stationary), refreshed each step by 8 PE transposes.
"""

import numpy as np

import concourse.bass as bass
from concourse import bacc
import concourse.mybir as mybir
import concourse.tile as tile
from concourse.bass_utils import run_bass_kernel_spmd
from concourse.masks import make_identity

B, SRC, TGT = 1024, 50, 25
I, H = 69, 1024
T = SRC + TGT - 1            # 74 frames
NCORES = 8
BL = B // NCORES             # 128 batch rows per core
IA = I + 1                   # 69 inputs + ones row (bias)
HK = H // 128                # 8 contraction chunks of h
G3 = 3 * H                   # 3072 gate columns (r|z|n)
NW = 512                     # burst width (one PSUM bank of fp32)

F32 = mybir.dt.float32
F32R = mybir.dt.float32r
AL = mybir.AluOpType
AF = mybir.ActivationFunctionType

LAST_RESULT = None           # BassKernelResults of the most recent run (for test.py)


def build_gru(nc, ins, outs, n_enc=T, n_dec=T):
    """Emit the full kernel into `nc`. `ins`/`outs` are dicts of DRAM APs."""
    # fp32r shares the fp32 byte layout; view f32-declared DRAM inputs as f32r
    # so the load DMAs are cast-free and verifier-visible as rounded producers.
    ins = {
        k: (v.bitcast(F32R) if k != "fc1b" and k != "fc2b" and v.dtype == F32 else v)
        for k, v in ins.items()
    }
    frames_d = ins["frames"]      # [T, IA, BL]
    whh_d = ins["whhT"]           # [HK, 128, G3]  whhT[c,k,n] = W_hh[n, c*128+k]
    wih_d = ins["wihT"]           # [IA, G3] rows 0:69 = W_ih^T, row 69 = bias row
    bhn_d = ins["bhn"]            # [1, H]   b_hh n-part
    fc1w_d = ins["fc1wT"]         # [HK, 128, I]
    fc1b_d = ins["fc1b"]          # [I, 1]
    fc2_d = ins["fc2T"]           # [IA, T]  rows 0:69 = per-step fc2 cols, row 69 = 0
    fc2b_d = ins["fc2b"]          # [1, 1]
    out_d = outs["out"]           # [1, BL]

    # burst name -> gate column offset; PSUM tag equals burst name
    cols = {"r0": 0, "r1": 512, "z0": 1024, "z1": 1536, "hn0": 2048, "hn1": 2560}
    border = ["r0", "hn0", "r1", "hn1", "z0", "z1"]

    with tile.TileContext(nc) as tc:
        with (
            tc.tile_pool(name="const", bufs=1) as const,
            tc.tile_pool(name="state", bufs=1) as state,
            tc.tile_pool(name="work", bufs=2) as work,
            tc.tile_pool(name="xin", bufs=4) as xin,
            tc.tile_pool(name="xout", bufs=3) as xop,
            tc.tile_pool(name="psum", bufs=1, space="PSUM") as psum,
        ):
            # ---- resident weights / constants ----
            whh_sb = const.tile([128, HK, G3], F32R)
            for c in range(HK):
                nc.sync.dma_start(whh_sb[:, c], whh_d[c])
            wih_sb = const.tile([IA, G3], F32R)
            nc.sync.dma_start(wih_sb, wih_d)
            bhn_sb = const.tile([1, H], F32R)
            nc.sync.dma_start(bhn_sb, bhn_d)
            fc1w_sb = const.tile([128, HK, I], F32R)
            for c in range(HK):
                nc.sync.dma_start(fc1w_sb[:, c], fc1w_d[c])
            fc1b_sb = const.tile([I, 1], F32)
            nc.sync.dma_start(fc1b_sb, fc1b_d)
            fc2_sb = const.tile([IA, T], F32R)
            nc.sync.dma_start(fc2_sb, fc2_d)
            fc2b_sb = const.tile([1, 1], F32)
            nc.sync.dma_start(fc2b_sb, fc2b_d)
            ones_sb = const.tile([1, BL], F32R)
            nc.sync.dma_start(ones_sb, ins["onesv"])
            ident_g = const.tile([128, 128], F32)
            make_identity(nc, ident_g)
            # transposes depending directly on the gpsimd-built identity would
            # carry waits on 3 distinct semaphores (> the 2-slot LDW limit);
            # route it through DVE so its dep folds into the DVE semaphore.
            ident = const.tile([128, 128], F32)
            nc.vector.tensor_copy(out=ident, in_=ident_g)

            # ---- recurrent state ----
            h_buf = state.tile([128, H], F32)   # [B, H]
            nc.vector.memset(h_buf, 0.0)
            # DVE memset can't encode f32r; tensor_copy with an f32r output is
            # the sanctioned rounding producer, so zero hT via a copy instead.
            hT = state.tile([128, H], F32R)      # chunk c at [:, c*128:(c+1)*128]
            nc.vector.tensor_copy(out=hT, in_=h_buf)
            acc = state.tile([1, BL], F32)      # fc2 logit accumulator

            def gru_step(x_sb, dec_idx):
                """One GRUCell step. x_sb: [IA, BL] sbuf tile (row 69 == 1.0).
                dec_idx: None for encode, else decode step index.
                Returns the new xout tile for decode steps."""
                g = {}
                for k in border:
                    g[k] = psum.tile([128, NW], F32, tag=k, name=f"g_{k}")
                gi = {
                    0: psum.tile([128, NW], F32, tag="pin0", name="gi_0"),
                    1: psum.tile([128, NW], F32, tag="pin1", name="gi_1"),
                }

                # x-phase: bias + input contributions (independent of new h)
                for k in border:
                    c0 = cols[k]
                    if k.startswith("hn"):
                        nc.tensor.matmul(
                            g[k], ones_sb, bhn_sb[:, c0 - 2 * H:c0 - 2 * H + NW],
                            start=True, stop=False)
                        nc.tensor.matmul(
                            gi[int(k[2])], x_sb, wih_sb[:, c0:c0 + NW],
                            start=True, stop=True)
                    else:
                        nc.tensor.matmul(
                            g[k], x_sb, wih_sb[:, c0:c0 + NW],
                            start=True, stop=False)

                # h-phase bursts + interleaved elementwise
                def hburst(k):
                    c0 = cols[k]
                    for c in range(HK):
                        nc.tensor.matmul(
                            g[k], hT[:, c * 128:(c + 1) * 128],
                            whh_sb[:, c, c0:c0 + NW],
                            start=False, stop=(c == HK - 1))

                r_sb, n_sb, hmn_sb, z_sb = {}, {}, {}, {}

                def ew_n(j):
                    # after r{j} and hn{j} bursts: n_j = tanh(i_n + r*h_n); hmn = h - n
                    rj = work.tile([128, NW], F32, tag=f"r{j}_sb", name=f"r{j}_sb")
                    nc.scalar.activation(rj, g[f"r{j}"], AF.Sigmoid)
                    r_sb[j] = rj
                    rh = work.tile([128, NW], F32, tag=f"rh{j}", name=f"rh{j}")
                    nc.vector.tensor_tensor(rh, rj, g[f"hn{j}"], AL.mult)
                    npre = work.tile([128, NW], F32, tag=f"np{j}", name=f"np{j}")
                    nc.vector.tensor_tensor(npre, rh, gi[j], AL.add)
                    nj = work.tile([128, NW], F32, tag=f"n{j}_sb", name=f"n{j}_sb")
                    nc.scalar.activation(nj, npre, AF.Tanh)
                    n_sb[j] = nj
                    hm = work.tile([128, NW], F32, tag=f"hmn{j}", name=f"hmn{j}")
                    nc.vector.tensor_tensor(
                        hm, h_buf[:, j * NW:(j + 1) * NW], nj, AL.subtract)
                    hmn_sb[j] = hm

                def ew_z(j):
                    # after z{j} burst: h_new_j = n + z*(h-n), written into h_buf
                    zj = work.tile([128, NW], F32, tag=f"z{j}_sb", name=f"z{j}_sb")
                    nc.scalar.activation(zj, g[f"z{j}"], AF.Sigmoid)
                    z_sb[j] = zj
                    zt = work.tile([128, NW], F32, tag=f"zt{j}", name=f"zt{j}")
                    nc.vector.tensor_tensor(zt, zj, hmn_sb[j], AL.mult)
                    nc.vector.tensor_tensor(
                        h_buf[:, j * NW:(j + 1) * NW], n_sb[j], zt, AL.add)

                hburst("r0")
                hburst("hn0")
                ew_n(0)
                hburst("r1")
                hburst("hn1")
                ew_n(1)
                hburst("z0")
                ew_z(0)
                hburst("z1")
                ew_z(1)

                # refresh hT: 8 PE transposes (4 per half) + copyback
                tp = {
                    0: psum.tile([128, NW], F32, tag="z0", name="tp0"),
                    1: psum.tile([128, NW], F32, tag="z1", name="tp1"),
                }
                for j in (0, 1):
                    for i in range(4):
                        c = 4 * j + i
                        nc.tensor.transpose(
                            tp[j][:, i * 128:(i + 1) * 128],
                            h_buf[:, c * 128:(c + 1) * 128], ident)
                for j in (0, 1):
                    nc.any.tensor_copy(out=hT[:, j * NW:(j + 1) * NW], in_=tp[j])

                if dec_idx is None:
                    return None

                # decode extras: out_d = x_d + h@fc1_w.T + fc1_b ; logit += out_d @ fc2_d
                fp = psum.tile([I, BL], F32, tag="pin0", name="fc1_ps")
                for c in range(HK):
                    nc.tensor.matmul(
                        fp, fc1w_sb[:, c], hT[:, c * 128:(c + 1) * 128],
                        start=(c == 0), stop=(c == HK - 1))
                xo = xop.tile([IA, BL], F32R, tag="xo", name="xo")
                # engines can't write partition-start 69 and DVE memset can't
                # encode f32r, so the constant row comes in by DMA.
                nc.sync.dma_start(xo[I:IA, :], ins["onesv"])
                nc.vector.scalar_tensor_tensor(
                    out=xo[0:I, :], in0=fp, scalar=fc1b_sb, in1=x_sb[0:I, :],
                    op0=AL.add, op1=AL.add)
                lp = psum.tile([1, BL], F32, tag="pin1", name="fc2_ps")
                nc.tensor.matmul(
                    lp, fc2_sb[:, dec_idx:dec_idx + 1], xo,
                    start=True, stop=True)
                if dec_idx == 0:
                    nc.vector.tensor_copy(out=acc, in_=lp)
                else:
                    nc.vector.tensor_tensor(acc, acc, lp, AL.add)
                return xo

            # ---- encode pass ----
            for t in range(n_enc):
                x_sb = xin.tile([IA, BL], F32R, tag="x", name="x")
                nc.sync.dma_start(x_sb, frames_d[t])
                gru_step(x_sb, None)

            # ---- decode pass ----
            x_sb = xin.tile([IA, BL], F32R, tag="x", name="x")
            nc.sync.dma_start(x_sb, frames_d[0])
            for d in range(n_dec):
                x_sb = gru_step(x_sb, d)

            # ---- final sigmoid + store ----
            res = state.tile([1, BL], F32)
            nc.scalar.activation(res, acc, AF.Sigmoid, bias=fc2b_sb[0:1, 0:1])
            nc.sync.dma_start(out_d, res)


def prep_inputs(inputs):
    """Host-side packing of the full-problem inputs into per-core DMA layouts."""
    enc = np.asarray(inputs["encoder_inputs"], np.float32)
    dec = np.asarray(inputs["decoder_output"], np.float32)
    w_ih = np.asarray(inputs["w_ih"], np.float32)
    w_hh = np.asarray(inputs["w_hh"], np.float32)
    b_ih = np.asarray(inputs["b_ih"], np.float32)
    b_hh = np.asarray(inputs["b_hh"], np.float32)
    fc1_w = np.asarray(inputs["fc1_w"], np.float32)
    fc1_b = np.asarray(inputs["fc1_b"], np.float32)
    fc2_w = np.asarray(inputs["fc2_w"], np.float32)
    fc2_b = np.asarray(inputs["fc2_b"], np.float32)

    all_frame = np.concatenate([enc, dec], axis=1)               # [B, T, I]
    framesT = all_frame.transpose(1, 2, 0)                       # [T, I, B]
    frames_aug = np.empty((T, IA, B), np.float32)
    frames_aug[:, :I] = framesT
    frames_aug[:, I] = 1.0

    whhT = np.ascontiguousarray(w_hh.T.reshape(HK, 128, G3))
    wihT = np.empty((IA, G3), np.float32)
    wihT[:I] = w_ih.T
    bsum = b_ih + b_hh
    wihT[I, :2 * H] = bsum[:2 * H]
    wihT[I, 2 * H:] = b_ih[2 * H:]
    bhn = np.ascontiguousarray(b_hh[2 * H:][None])               # [1, H]
    fc1wT = np.ascontiguousarray(fc1_w.T.reshape(HK, 128, I))
    fc1b = np.ascontiguousarray(fc1_b[:, None])
    fc2T = np.zeros((IA, T), np.float32)
    fc2T[:I] = fc2_w.reshape(T, I).T
    fc2b = np.asarray(fc2_b, np.float32).reshape(1, 1)

    shared = {
        "whhT": whhT, "wihT": wihT, "bhn": bhn, "fc1wT": fc1wT,
        "fc1b": fc1b, "fc2T": fc2T, "fc2b": fc2b,
        "onesv": np.ones((1, BL), np.float32),
    }
    in_maps = []
    for k in range(NCORES):
        m = dict(shared)
        m["frames"] = np.ascontiguousarray(frames_aug[:, :, k * BL:(k + 1) * BL])
        in_maps.append(m)
    return in_maps


def declare_io(nc):
    aps = {
        "frames": nc.dram_tensor("frames", [T, IA, BL], F32R, kind="ExternalInput").ap(),
        "whhT": nc.dram_tensor("whhT", [HK, 128, G3], F32R, kind="ExternalInput").ap(),
        "wihT": nc.dram_tensor("wihT", [IA, G3], F32R, kind="ExternalInput").ap(),
        "bhn": nc.dram_tensor("bhn", [1, H], F32R, kind="ExternalInput").ap(),
        "fc1wT": nc.dram_tensor("fc1wT", [HK, 128, I], F32R, kind="ExternalInput").ap(),
        "fc1b": nc.dram_tensor("fc1b", [I, 1], F32, kind="ExternalInput").ap(),
        "fc2T": nc.dram_tensor("fc2T", [IA, T], F32R, kind="ExternalInput").ap(),
        "fc2b": nc.dram_tensor("fc2b", [1, 1], F32, kind="ExternalInput").ap(),
        "onesv": nc.dram_tensor("onesv", [1, BL], F32R, kind="ExternalInput").ap(),
    }
    out_ap = nc.dram_tensor("out", [1, BL], F32, kind="ExternalOutput").ap()
    return aps, out_ap


def kernel(**inputs) -> np.ndarray:
    global LAST_RESULT
    in_maps = prep_inputs(inputs)

    nc = bacc.Bacc("TRN2", num_devices=NCORES, enable_asserts=False)
    aps, out_ap = declare_io(nc)
    build_gru(nc, aps, {"out": out_ap})
    nc.finalize()

    LAST_RESULT = run_bass_kernel_spmd(nc, in_maps, core_ids=list(range(NCORES)))

    out = np.empty((B, 1), np.float32)
    for k in range(NCORES):
        out[k * BL:(k + 1) * BL, 0] = LAST_RESULT.results[k]["out"][0]
    return out


# revision 13
# speedup vs baseline: 4386.3509x; 4386.3509x over previous
"""Trainium2 Bass kernel: DiscriminatorRNN (GRU encode + autoregressive GRU decode).

Math (per reference):
  frames = concat(encoder_inputs, decoder_output) transposed to [T,B,I], T=74
  encode: h = GRUCell(frames[t], h)  for t in 0..73   (h0 = 0)
  decode: x0 = frames[0]; for d in 0..73: h = GRUCell(x_d, h); out_d = x_d + h@fc1_w.T + fc1_b; x_{d+1} = out_d
  logit[b] = sum_{d,i} out_d[b,i] * fc2_w[0, d*69+i] + fc2_b;  result = sigmoid(logit)  -> [B,1]

Distribution: pure data-parallel over batch, B=1024 -> 128 rows per core on 8 cores,
weights replicated, zero communication.

Per-core layout (one NeuronCore):
  - batch rows live on the matmul *stationary* operand (M=128), the GRU weights are
    the *moving* operand (float32r at full PE rate for free-dim >= 256).
  - gates g = x@Wih^T + h@Whh^T + biases accumulate in PSUM as [B=128, 3H] in six
    512-wide bursts (r0 r1 | z0 z1 | n0 n1), i_n and h_n kept in separate banks.
  - x is augmented with a constant 1.0 row; Wih^T gets a bias row, so biases ride the
    x matmul. h_n's b_hh bias comes from a K=1 ones-outer-product matmul.
  - the recurrent state is kept both as h [B,H] (for the elementwise update) and as
    h^T chunks [H,B] (the matmul stationary), refreshed each step by 8 PE transposes.
"""

import numpy as np

import concourse.bass as bass
from concourse import bacc
import concourse.mybir as mybir
import concourse.tile as tile
from concourse.bass_utils import run_bass_kernel_spmd
from concourse.masks import make_identity

B, SRC, TGT = 1024, 50, 25
I, H = 69, 1024
T = SRC + TGT - 1            # 74 frames
NCORES = 8
BL = B // NCORES             # 128 batch rows per core
IA = I + 1                   # 69 inputs + ones row (bias)
HK = H // 128                # 8 contraction chunks of h
G3 = 3 * H                   # 3072 gate columns (r|z|n)
NW = 512                     # burst width (one PSUM bank of fp32)

F32 = mybir.dt.float32
F32R = mybir.dt.float32r
BF16 = mybir.dt.bfloat16
AL = mybir.AluOpType
AF = mybir.ActivationFunctionType

LAST_RESULT = None           # BassKernelResults of the most recent run (for test.py)


def build_gru(nc, ins, outs, n_enc=T, n_dec=T):
    """Emit the full kernel into `nc`. `ins`/`outs` are dicts of DRAM APs."""
    # fp32r shares the fp32 byte layout; view f32-declared DRAM inputs as f32r
    # so the load DMAs are cast-free and verifier-visible as rounded producers.
    ins = {
        k: (v.bitcast(F32R) if k != "fc1b" and k != "fc2b" and v.dtype == F32 else v)
        for k, v in ins.items()
    }
    frames_d = ins["frames"]      # [T, IA, BL]
    whh_d = ins["whhT"]           # [HK, 128, G3]  whhT[c,k,n] = W_hh[n, c*128+k]
    wih_d = ins["wihT"]           # [IA, G3] rows 0:69 = W_ih^T, row 69 = bias row
    bhn_d = ins["bhn"]            # [1, H]   b_hh n-part
    fc1w_d = ins["fc1wT"]         # [HK, 128, I]
    fc1b_d = ins["fc1b"]          # [I, 1]
    fc2_d = ins["fc2T"]           # [IA, T]  rows 0:69 = per-step fc2 cols, row 69 = 0
    fc2b_d = ins["fc2b"]          # [1, 1]
    out_d = outs["out"]           # [1, BL]

    # burst name -> gate column offset; PSUM tag equals burst name
    cols = {"r0": 0, "r1": 512, "z0": 1024, "z1": 1536, "hn0": 2048, "hn1": 2560}
    border = ["r0", "hn0", "r1", "hn1", "z0", "z1"]

    with tile.TileContext(nc) as tc:
        with (
            tc.tile_pool(name="const", bufs=1) as const,
            tc.tile_pool(name="state", bufs=1) as state,
            tc.tile_pool(name="work", bufs=2) as work,
            tc.tile_pool(name="xin", bufs=4) as xin,
            tc.tile_pool(name="xout", bufs=3) as xop,
            tc.tile_pool(name="psum", bufs=1, space="PSUM") as psum,
        ):
            # ---- resident weights / constants ----
            whh_sb = const.tile([128, HK, G3], F32R)
            for c in range(HK):
                nc.sync.dma_start(whh_sb[:, c], whh_d[c])
            wih_sb = const.tile([IA, G3], F32R)
            nc.sync.dma_start(wih_sb, wih_d)
            bhn_sb = const.tile([1, H], F32R)
            nc.sync.dma_start(bhn_sb, bhn_d)
            fc1w_sb = const.tile([128, HK, I], F32R)
            for c in range(HK):
                nc.sync.dma_start(fc1w_sb[:, c], fc1w_d[c])
            fc1b_sb = const.tile([I, 1], F32)
            nc.sync.dma_start(fc1b_sb, fc1b_d)
            # bf16 copy of fc1 weights: N=128 fp32r matmuls run at 1/4 rate,
            # bf16 runs full rate; fc1's contribution tolerates bf16.
            fc1w_b = const.tile([128, HK, I], BF16)
            nc.vector.tensor_copy(out=fc1w_b, in_=fc1w_sb)
            fc2_sb = const.tile([IA, T], F32R)
            nc.sync.dma_start(fc2_sb, fc2_d)
            fc2b_sb = const.tile([1, 1], F32)
            nc.sync.dma_start(fc2b_sb, fc2b_d)
            ones_sb = const.tile([1, BL], F32R)
            nc.sync.dma_start(ones_sb, ins["onesv"])
            ident_g = const.tile([128, 128], F32)
            make_identity(nc, ident_g)
            # transposes depending directly on the gpsimd-built identity would
            # carry waits on 3 distinct semaphores (> the 2-slot LDW limit);
            # route it through DVE so its dep folds into the DVE semaphore.
            ident = const.tile([128, 128], F32)
            nc.vector.tensor_copy(out=ident, in_=ident_g)

            # ---- recurrent state ----
            h_buf = state.tile([128, H], F32)   # [B, H]
            nc.vector.memset(h_buf, 0.0)
            # DVE memset can't encode f32r; tensor_copy with an f32r output is
            # the sanctioned rounding producer, so zero hT via a copy instead.
            hT = state.tile([128, H], F32R)      # chunk c at [:, c*128:(c+1)*128]
            nc.vector.tensor_copy(out=hT, in_=h_buf)
            hT_b = state.tile([128, H], BF16)    # bf16 twin, feeds fc1 (decode)
            nc.vector.tensor_copy(out=hT_b, in_=h_buf)
            acc = state.tile([1, BL], F32)      # fc2 logit accumulator

            def gru_step(x_sb, dec_idx):
                """One GRUCell step. x_sb: [IA, BL] sbuf tile (row 69 == 1.0).
                dec_idx: None for encode, else decode step index.
                Returns the new xout tile for decode steps."""
                g = {}
                for k in border:
                    g[k] = psum.tile([128, NW], F32, tag=k, name=f"g_{k}")
                gi = {
                    0: psum.tile([128, NW], F32, tag="pin0", name="gi_0"),
                    1: psum.tile([128, NW], F32, tag="pin1", name="gi_1"),
                }

                # x-phase: bias + input contributions (independent of new h)
                for k in border:
                    c0 = cols[k]
                    if k.startswith("hn"):
                        nc.tensor.matmul(
                            g[k], ones_sb, bhn_sb[:, c0 - 2 * H:c0 - 2 * H + NW],
                            start=True, stop=False)
                        nc.tensor.matmul(
                            gi[int(k[2])], x_sb, wih_sb[:, c0:c0 + NW],
                            start=True, stop=True)
                    else:
                        nc.tensor.matmul(
                            g[k], x_sb, wih_sb[:, c0:c0 + NW],
                            start=True, stop=False)

                # h-phase bursts + interleaved elementwise
                def hburst(k):
                    c0 = cols[k]
                    for c in range(HK):
                        nc.tensor.matmul(
                            g[k], hT[:, c * 128:(c + 1) * 128],
                            whh_sb[:, c, c0:c0 + NW],
                            start=False, stop=(c == HK - 1))

                r_sb, n_sb, hmn_sb, z_sb = {}, {}, {}, {}

                def ew_n(j):
                    # after r{j} and hn{j} bursts: n_j = tanh(i_n + r*h_n); hmn = h - n
                    rj = work.tile([128, NW], F32, tag=f"r{j}_sb", name=f"r{j}_sb")
                    nc.scalar.activation(rj, g[f"r{j}"], AF.Sigmoid)
                    r_sb[j] = rj
                    rh = work.tile([128, NW], F32, tag=f"rh{j}", name=f"rh{j}")
                    nc.vector.tensor_tensor(rh, rj, g[f"hn{j}"], AL.mult)
                    npre = work.tile([128, NW], F32, tag=f"np{j}", name=f"np{j}")
                    nc.vector.tensor_tensor(npre, rh, gi[j], AL.add)
                    nj = work.tile([128, NW], F32, tag=f"n{j}_sb", name=f"n{j}_sb")
                    nc.scalar.activation(nj, npre, AF.Tanh)
                    n_sb[j] = nj
                    hm = work.tile([128, NW], F32, tag=f"hmn{j}", name=f"hmn{j}")
                    nc.vector.tensor_tensor(
                        hm, h_buf[:, j * NW:(j + 1) * NW], nj, AL.subtract)
                    hmn_sb[j] = hm

                def ew_z(j):
                    # after z{j} burst: h_new_j = n + z*(h-n), written into h_buf
                    zj = work.tile([128, NW], F32, tag=f"z{j}_sb", name=f"z{j}_sb")
                    nc.scalar.activation(zj, g[f"z{j}"], AF.Sigmoid)
                    z_sb[j] = zj
                    zt = work.tile([128, NW], F32, tag=f"zt{j}", name=f"zt{j}")
                    nc.vector.tensor_tensor(zt, zj, hmn_sb[j], AL.mult)
                    nc.vector.tensor_tensor(
                        h_buf[:, j * NW:(j + 1) * NW], n_sb[j], zt, AL.add)

                # h-phase in two chunk-half passes: pass A (chunks 0..3) only
                # needs the first half of hT, which the previous step's tail
                # produces ~2us before the second half -- so this step's A-pass
                # overlaps the previous step's elementwise tail.
                for c in range(4):
                    for k in border:
                        c0 = cols[k]
                        nc.tensor.matmul(
                            g[k], hT[:, c * 128:(c + 1) * 128],
                            whh_sb[:, c, c0:c0 + NW],
                            start=False, stop=False)
                for k in border:
                    c0 = cols[k]
                    for c in range(4, HK):
                        nc.tensor.matmul(
                            g[k], hT[:, c * 128:(c + 1) * 128],
                            whh_sb[:, c, c0:c0 + NW],
                            start=False, stop=(c == HK - 1))
                    if k == "hn0":
                        ew_n(0)
                    elif k == "hn1":
                        ew_n(1)
                    elif k == "z0":
                        ew_z(0)
                    elif k == "z1":
                        ew_z(1)

                # refresh hT: 8 PE transposes (4 per half) + copyback
                tp = {
                    0: psum.tile([128, NW], F32, tag="z0", name="tp0"),
                    1: psum.tile([128, NW], F32, tag="z1", name="tp1"),
                }
                for j in (0, 1):
                    for i in range(4):
                        c = 4 * j + i
                        nc.tensor.transpose(
                            tp[j][:, i * 128:(i + 1) * 128],
                            h_buf[:, c * 128:(c + 1) * 128], ident)
                for j in (0, 1):
                    nc.any.tensor_copy(out=hT[:, j * NW:(j + 1) * NW], in_=tp[j])

                if dec_idx is None:
                    return None
                for j in (0, 1):
                    nc.any.tensor_copy(out=hT_b[:, j * NW:(j + 1) * NW], in_=tp[j])

                # decode extras: out_d = x_d + h@fc1_w.T + fc1_b ; logit += out_d @ fc2_d
                fp = psum.tile([I, BL], F32, tag="pin0", name="fc1_ps")
                for c in range(HK):
                    nc.tensor.matmul(
                        fp, fc1w_b[:, c], hT_b[:, c * 128:(c + 1) * 128],
                        start=(c == 0), stop=(c == HK - 1))
                xo = xop.tile([IA, BL], F32R, tag="xo", name="xo")
                # engines can't write partition-start 69 and DVE memset can't
                # encode f32r, so the constant row comes in by DMA.
                nc.sync.dma_start(xo[I:IA, :], ins["onesv"])
                nc.vector.scalar_tensor_tensor(
                    out=xo[0:I, :], in0=fp, scalar=fc1b_sb, in1=x_sb[0:I, :],
                    op0=AL.add, op1=AL.add)
                lp = psum.tile([1, BL], F32, tag="pin1", name="fc2_ps")
                nc.tensor.matmul(
                    lp, fc2_sb[:, dec_idx:dec_idx + 1], xo,
                    start=True, stop=True)
                if dec_idx == 0:
                    nc.vector.tensor_copy(out=acc, in_=lp)
                else:
                    nc.vector.tensor_tensor(acc, acc, lp, AL.add)
                return xo

            # ---- encode pass ----
            for t in range(n_enc):
                x_sb = xin.tile([IA, BL], F32R, tag="x", name="x")
                nc.sync.dma_start(x_sb, frames_d[t])
                gru_step(x_sb, None)

            # ---- decode pass ----
            x_sb = xin.tile([IA, BL], F32R, tag="x", name="x")
            nc.sync.dma_start(x_sb, frames_d[0])
            for d in range(n_dec):
                x_sb = gru_step(x_sb, d)

            # ---- final sigmoid + store ----
            res = state.tile([1, BL], F32)
            nc.scalar.activation(res, acc, AF.Sigmoid, bias=fc2b_sb[0:1, 0:1])
            nc.sync.dma_start(out_d, res)


def prep_inputs(inputs):
    """Host-side packing of the full-problem inputs into per-core DMA layouts."""
    enc = np.asarray(inputs["encoder_inputs"], np.float32)
    dec = np.asarray(inputs["decoder_output"], np.float32)
    w_ih = np.asarray(inputs["w_ih"], np.float32)
    w_hh = np.asarray(inputs["w_hh"], np.float32)
    b_ih = np.asarray(inputs["b_ih"], np.float32)
    b_hh = np.asarray(inputs["b_hh"], np.float32)
    fc1_w = np.asarray(inputs["fc1_w"], np.float32)
    fc1_b = np.asarray(inputs["fc1_b"], np.float32)
    fc2_w = np.asarray(inputs["fc2_w"], np.float32)
    fc2_b = np.asarray(inputs["fc2_b"], np.float32)

    all_frame = np.concatenate([enc, dec], axis=1)               # [B, T, I]
    framesT = all_frame.transpose(1, 2, 0)                       # [T, I, B]
    frames_aug = np.empty((T, IA, B), np.float32)
    frames_aug[:, :I] = framesT
    frames_aug[:, I] = 1.0

    whhT = np.ascontiguousarray(w_hh.T.reshape(HK, 128, G3))
    wihT = np.empty((IA, G3), np.float32)
    wihT[:I] = w_ih.T
    bsum = b_ih + b_hh
    wihT[I, :2 * H] = bsum[:2 * H]
    wihT[I, 2 * H:] = b_ih[2 * H:]
    bhn = np.ascontiguousarray(b_hh[2 * H:][None])               # [1, H]
    fc1wT = np.ascontiguousarray(fc1_w.T.reshape(HK, 128, I))
    fc1b = np.ascontiguousarray(fc1_b[:, None])
    fc2T = np.zeros((IA, T), np.float32)
    fc2T[:I] = fc2_w.reshape(T, I).T
    fc2b = np.asarray(fc2_b, np.float32).reshape(1, 1)

    shared = {
        "whhT": whhT, "wihT": wihT, "bhn": bhn, "fc1wT": fc1wT,
        "fc1b": fc1b, "fc2T": fc2T, "fc2b": fc2b,
        "onesv": np.ones((1, BL), np.float32),
    }
    in_maps = []
    for k in range(NCORES):
        m = dict(shared)
        m["frames"] = np.ascontiguousarray(frames_aug[:, :, k * BL:(k + 1) * BL])
        in_maps.append(m)
    return in_maps


def declare_io(nc):
    aps = {
        "frames": nc.dram_tensor("frames", [T, IA, BL], F32R, kind="ExternalInput").ap(),
        "whhT": nc.dram_tensor("whhT", [HK, 128, G3], F32R, kind="ExternalInput").ap(),
        "wihT": nc.dram_tensor("wihT", [IA, G3], F32R, kind="ExternalInput").ap(),
        "bhn": nc.dram_tensor("bhn", [1, H], F32R, kind="ExternalInput").ap(),
        "fc1wT": nc.dram_tensor("fc1wT", [HK, 128, I], F32R, kind="ExternalInput").ap(),
        "fc1b": nc.dram_tensor("fc1b", [I, 1], F32, kind="ExternalInput").ap(),
        "fc2T": nc.dram_tensor("fc2T", [IA, T], F32R, kind="ExternalInput").ap(),
        "fc2b": nc.dram_tensor("fc2b", [1, 1], F32, kind="ExternalInput").ap(),
        "onesv": nc.dram_tensor("onesv", [1, BL], F32R, kind="ExternalInput").ap(),
    }
    out_ap = nc.dram_tensor("out", [1, BL], F32, kind="ExternalOutput").ap()
    return aps, out_ap


def kernel(**inputs) -> np.ndarray:
    global LAST_RESULT
    in_maps = prep_inputs(inputs)

    nc = bacc.Bacc("TRN2", num_devices=NCORES, enable_asserts=False)
    aps, out_ap = declare_io(nc)
    build_gru(nc, aps, {"out": out_ap})
    nc.finalize()

    LAST_RESULT = run_bass_kernel_spmd(nc, in_maps, core_ids=list(range(NCORES)))

    out = np.empty((B, 1), np.float32)
    for k in range(NCORES):
        out[k * BL:(k + 1) * BL, 0] = LAST_RESULT.results[k]["out"][0]
    return out


# revision 16
# speedup vs baseline: 5249.9762x; 1.1969x over previous
"""Trainium2 Bass kernel: DiscriminatorRNN (GRU encode + autoregressive GRU decode).

Math (per reference):
  frames = concat(encoder_inputs, decoder_output) transposed to [T,B,I], T=74
  encode: h = GRUCell(frames[t], h)  for t in 0..73   (h0 = 0)
  decode: x0 = frames[0]; for d in 0..73: h = GRUCell(x_d, h); out_d = x_d + h@fc1_w.T + fc1_b; x_{d+1} = out_d
  logit[b] = sum_{d,i} out_d[b,i] * fc2_w[0, d*69+i] + fc2_b;  result = sigmoid(logit)  -> [B,1]

Distribution: pure data-parallel over batch, B=1024 -> 128 rows per core on 8 cores,
weights replicated, zero communication.

Per-core layout (one NeuronCore):
  - batch rows live on the matmul *stationary* operand (M=128), the GRU weights are
    the *moving* operand (float32r at full PE rate for free-dim >= 256).
  - gates g = x@Wih^T + h@Whh^T + biases accumulate in PSUM as [B=128, 3H] in six
    512-wide bursts (r0 r1 | z0 z1 | n0 n1), i_n and h_n kept in separate banks.
  - x is augmented with a constant 1.0 row; Wih^T gets a bias row, so biases ride the
    x matmul. h_n's b_hh bias comes from a K=1 ones-outer-product matmul.
  - the recurrent state is kept both as h [B,H] (for the elementwise update) and as
    h^T chunks [H,B] (the matmul stationary), refreshed each step by 8 PE transposes.
"""

import numpy as np

import concourse.bass as bass
from concourse import bacc
import concourse.mybir as mybir
import concourse.tile as tile
from concourse.bass_utils import run_bass_kernel_spmd
from concourse.masks import make_identity

B, SRC, TGT = 1024, 50, 25
I, H = 69, 1024
T = SRC + TGT - 1            # 74 frames
NCORES = 8
BL = B // NCORES             # 128 batch rows per core
IA = I + 1                   # 69 inputs + ones row (bias)
HK = H // 128                # 8 contraction chunks of h
G3 = 3 * H                   # 3072 gate columns (r|z|n)
NW = 512                     # burst width (one PSUM bank of fp32)

F32 = mybir.dt.float32
F32R = mybir.dt.float32r
BF16 = mybir.dt.bfloat16
AL = mybir.AluOpType
AF = mybir.ActivationFunctionType

LAST_RESULT = None           # BassKernelResults of the most recent run (for test.py)


def build_gru(nc, ins, outs, n_enc=T, n_dec=T):
    """Emit the full kernel into `nc`. `ins`/`outs` are dicts of DRAM APs."""
    # fp32r shares the fp32 byte layout; view f32-declared DRAM inputs as f32r
    # so the load DMAs are cast-free and verifier-visible as rounded producers.
    ins = {
        k: (v.bitcast(F32R) if k != "fc1b" and k != "fc2b" and v.dtype == F32 else v)
        for k, v in ins.items()
    }
    frames_d = ins["frames"]      # [T, IA, BL]
    whh_d = ins["whhT"]           # [HK, 128, G3]  whhT[c,k,n] = W_hh[n, c*128+k]
    wih_d = ins["wihT"]           # [IA, G3] rows 0:69 = W_ih^T, row 69 = bias row
    bhn_d = ins["bhn"]            # [1, H]   b_hh n-part
    fc1w_d = ins["fc1wT"]         # [HK, 128, I]
    fc1b_d = ins["fc1b"]          # [I, 1]
    fc2_d = ins["fc2T"]           # [IA, T]  rows 0:69 = per-step fc2 cols, row 69 = 0
    fc2b_d = ins["fc2b"]          # [1, 1]
    out_d = outs["out"]           # [1, BL]

    # burst name -> gate column offset; PSUM tag equals burst name
    cols = {"r0": 0, "r1": 512, "z0": 1024, "z1": 1536, "hn0": 2048, "hn1": 2560}
    border = ["r0", "hn0", "r1", "hn1", "z0", "z1"]

    with tile.TileContext(nc) as tc:
        with (
            tc.tile_pool(name="const", bufs=1) as const,
            tc.tile_pool(name="state", bufs=1) as state,
            tc.tile_pool(name="work", bufs=2) as work,
            tc.tile_pool(name="xin", bufs=4) as xin,
            tc.tile_pool(name="xout", bufs=3) as xop,
            tc.tile_pool(name="psum", bufs=1, space="PSUM") as psum,
        ):
            # ---- resident weights / constants ----
            whh_sb = const.tile([128, HK, G3], F32R)
            for c in range(HK):
                nc.sync.dma_start(whh_sb[:, c], whh_d[c])
            wih_sb = const.tile([IA, G3], F32R)
            nc.sync.dma_start(wih_sb, wih_d)
            bhn_sb = const.tile([1, H], F32R)
            nc.sync.dma_start(bhn_sb, bhn_d)
            fc1w_sb = const.tile([128, HK, I], F32R)
            for c in range(HK):
                nc.sync.dma_start(fc1w_sb[:, c], fc1w_d[c])
            fc1b_sb = const.tile([I, 1], F32)
            nc.sync.dma_start(fc1b_sb, fc1b_d)
            # bf16 copy of fc1 weights: N=128 fp32r matmuls run at 1/4 rate,
            # bf16 runs full rate; fc1's contribution tolerates bf16.
            fc1w_b = const.tile([128, HK, I], BF16)
            nc.vector.tensor_copy(out=fc1w_b, in_=fc1w_sb)
            fc2_sb = const.tile([IA, T], F32R)
            nc.sync.dma_start(fc2_sb, fc2_d)
            fc2b_sb = const.tile([1, 1], F32)
            nc.sync.dma_start(fc2b_sb, fc2b_d)
            ones_sb = const.tile([1, BL], F32R)
            nc.sync.dma_start(ones_sb, ins["onesv"])
            ident_g = const.tile([128, 128], F32)
            make_identity(nc, ident_g)
            # transposes depending directly on the gpsimd-built identity would
            # carry waits on 3 distinct semaphores (> the 2-slot LDW limit);
            # route it through DVE so its dep folds into the DVE semaphore.
            ident = const.tile([128, 128], F32)
            nc.vector.tensor_copy(out=ident, in_=ident_g)

            # ---- recurrent state ----
            h_buf = state.tile([128, H], F32)   # [B, H]
            nc.vector.memset(h_buf, 0.0)
            # DVE memset can't encode f32r; tensor_copy with an f32r output is
            # the sanctioned rounding producer, so zero hT via a copy instead.
            hT = state.tile([128, H], F32R)      # chunk c at [:, c*128:(c+1)*128]
            nc.vector.tensor_copy(out=hT, in_=h_buf)
            hT_b = state.tile([128, H], BF16)    # bf16 twin, feeds fc1 (decode)
            nc.vector.tensor_copy(out=hT_b, in_=h_buf)
            acc = state.tile([1, BL], F32)      # fc2 logit accumulator

            def gru_step(x_sb, dec_idx):
                """One GRUCell step. x_sb: [IA, BL] sbuf tile (row 69 == 1.0).
                dec_idx: None for encode, else decode step index.
                Returns the new xout tile for decode steps."""
                g = {}
                for k in border:
                    g[k] = psum.tile([128, NW], F32, tag=k, name=f"g_{k}")
                gi = {
                    0: psum.tile([128, NW], F32, tag="pin0", name="gi_0"),
                    1: psum.tile([128, NW], F32, tag="pin1", name="gi_1"),
                }

                # On encode steps x arrives early by DMA: emit its matmuls first
                # (they prefetch into the previous step's tail). On decode steps
                # x is the previous step's fc1 output and lands *after* the
                # copybacks, so its matmuls go after the h-bursts instead and the
                # h-matmuls open the PSUM accumulation groups.
                late_x = dec_idx is not None

                def x_mm(k):
                    c0 = cols[k]
                    if k.startswith("hn"):
                        nc.tensor.matmul(
                            gi[int(k[2])], x_sb, wih_sb[:, c0:c0 + NW],
                            start=True, stop=True)
                    else:
                        nc.tensor.matmul(
                            g[k], x_sb, wih_sb[:, c0:c0 + NW],
                            start=not late_x, stop=late_x)

                for k in border:
                    c0 = cols[k]
                    if k.startswith("hn"):
                        # bias opener for the h_n banks (x-independent)
                        nc.tensor.matmul(
                            g[k], ones_sb, bhn_sb[:, c0 - 2 * H:c0 - 2 * H + NW],
                            start=True, stop=False)
                    if not late_x:
                        x_mm(k)

                # h-phase bursts + interleaved elementwise
                def hburst(k):
                    c0 = cols[k]
                    for c in range(HK):
                        nc.tensor.matmul(
                            g[k], hT[:, c * 128:(c + 1) * 128],
                            whh_sb[:, c, c0:c0 + NW],
                            start=(late_x and c == 0 and not k.startswith("hn")),
                            stop=(c == HK - 1 and (not late_x or k.startswith("hn"))))
                    if late_x:
                        x_mm(k)

                r_sb, n_sb, hmn_sb, z_sb = {}, {}, {}, {}

                def ew_n(j):
                    # after r{j} and hn{j} bursts: n_j = tanh(i_n + r*h_n); hmn = h - n
                    rj = work.tile([128, NW], F32, tag=f"r{j}_sb", name=f"r{j}_sb")
                    nc.scalar.activation(rj, g[f"r{j}"], AF.Sigmoid)
                    r_sb[j] = rj
                    rh = work.tile([128, NW], F32, tag=f"rh{j}", name=f"rh{j}")
                    nc.vector.tensor_tensor(rh, rj, g[f"hn{j}"], AL.mult)
                    npre = work.tile([128, NW], F32, tag=f"np{j}", name=f"np{j}")
                    nc.vector.tensor_tensor(npre, rh, gi[j], AL.add)
                    nj = work.tile([128, NW], F32, tag=f"n{j}_sb", name=f"n{j}_sb")
                    nc.scalar.activation(nj, npre, AF.Tanh)
                    n_sb[j] = nj
                    hm = work.tile([128, NW], F32, tag=f"hmn{j}", name=f"hmn{j}")
                    nc.vector.tensor_tensor(
                        hm, h_buf[:, j * NW:(j + 1) * NW], nj, AL.subtract)
                    hmn_sb[j] = hm

                def ew_z(j):
                    # after z{j} burst: h_new_j = n + z*(h-n), written into h_buf
                    zj = work.tile([128, NW], F32, tag=f"z{j}_sb", name=f"z{j}_sb")
                    nc.scalar.activation(zj, g[f"z{j}"], AF.Sigmoid)
                    z_sb[j] = zj
                    zt = work.tile([128, NW], F32, tag=f"zt{j}", name=f"zt{j}")
                    nc.vector.tensor_tensor(zt, zj, hmn_sb[j], AL.mult)
                    nc.vector.tensor_tensor(
                        h_buf[:, j * NW:(j + 1) * NW], n_sb[j], zt, AL.add)

                hburst("r0")
                hburst("hn0")
                ew_n(0)
                hburst("r1")
                hburst("hn1")
                ew_n(1)
                hburst("z0")
                ew_z(0)
                hburst("z1")
                ew_z(1)

                # refresh hT: 8 PE transposes (4 per half) + copyback
                tp = {
                    0: psum.tile([128, NW], F32, tag="z0", name="tp0"),
                    1: psum.tile([128, NW], F32, tag="z1", name="tp1"),
                }
                for j in (0, 1):
                    for i in range(4):
                        c = 4 * j + i
                        nc.tensor.transpose(
                            tp[j][:, i * 128:(i + 1) * 128],
                            h_buf[:, c * 128:(c + 1) * 128], ident)
                for j in (0, 1):
                    nc.any.tensor_copy(out=hT[:, j * NW:(j + 1) * NW], in_=tp[j])

                if dec_idx is None:
                    return None
                for j in (0, 1):
                    nc.any.tensor_copy(out=hT_b[:, j * NW:(j + 1) * NW], in_=tp[j])

                # decode extras: out_d = x_d + h@fc1_w.T + fc1_b ; logit += out_d @ fc2_d
                fp = psum.tile([I, BL], F32, tag="pin0", name="fc1_ps")
                for c in range(HK):
                    nc.tensor.matmul(
                        fp, fc1w_b[:, c], hT_b[:, c * 128:(c + 1) * 128],
                        start=(c == 0), stop=(c == HK - 1))
                xo = xop.tile([IA, BL], F32R, tag="xo", name="xo")
                # engines can't write partition-start 69 and DVE memset can't
                # encode f32r, so the constant row comes in by DMA.
                nc.sync.dma_start(xo[I:IA, :], ins["onesv"])
                nc.vector.scalar_tensor_tensor(
                    out=xo[0:I, :], in0=fp, scalar=fc1b_sb, in1=x_sb[0:I, :],
                    op0=AL.add, op1=AL.add)
                lp = psum.tile([1, BL], F32, tag="pin1", name="fc2_ps")
                nc.tensor.matmul(
                    lp, fc2_sb[:, dec_idx:dec_idx + 1], xo,
                    start=True, stop=True)
                if dec_idx == 0:
                    nc.vector.tensor_copy(out=acc, in_=lp)
                else:
                    nc.vector.tensor_tensor(acc, acc, lp, AL.add)
                return xo

            # ---- encode pass ----
            for t in range(n_enc):
                x_sb = xin.tile([IA, BL], F32R, tag="x", name="x")
                nc.sync.dma_start(x_sb, frames_d[t])
                gru_step(x_sb, None)

            # ---- decode pass ----
            x_sb = xin.tile([IA, BL], F32R, tag="x", name="x")
            nc.sync.dma_start(x_sb, frames_d[0])
            for d in range(n_dec):
                x_sb = gru_step(x_sb, d)

            # ---- final sigmoid + store ----
            res = state.tile([1, BL], F32)
            nc.scalar.activation(res, acc, AF.Sigmoid, bias=fc2b_sb[0:1, 0:1])
            nc.sync.dma_start(out_d, res)


def prep_inputs(inputs):
    """Host-side packing of the full-problem inputs into per-core DMA layouts."""
    enc = np.asarray(inputs["encoder_inputs"], np.float32)
    dec = np.asarray(inputs["decoder_output"], np.float32)
    w_ih = np.asarray(inputs["w_ih"], np.float32)
    w_hh = np.asarray(inputs["w_hh"], np.float32)
    b_ih = np.asarray(inputs["b_ih"], np.float32)
    b_hh = np.asarray(inputs["b_hh"], np.float32)
    fc1_w = np.asarray(inputs["fc1_w"], np.float32)
    fc1_b = np.asarray(inputs["fc1_b"], np.float32)
    fc2_w = np.asarray(inputs["fc2_w"], np.float32)
    fc2_b = np.asarray(inputs["fc2_b"], np.float32)

    all_frame = np.concatenate([enc, dec], axis=1)               # [B, T, I]
    framesT = all_frame.transpose(1, 2, 0)                       # [T, I, B]
    frames_aug = np.empty((T, IA, B), np.float32)
    frames_aug[:, :I] = framesT
    frames_aug[:, I] = 1.0

    whhT = np.ascontiguousarray(w_hh.T.reshape(HK, 128, G3))
    wihT = np.empty((IA, G3), np.float32)
    wihT[:I] = w_ih.T
    bsum = b_ih + b_hh
    wihT[I, :2 * H] = bsum[:2 * H]
    wihT[I, 2 * H:] = b_ih[2 * H:]
    bhn = np.ascontiguousarray(b_hh[2 * H:][None])               # [1, H]
    fc1wT = np.ascontiguousarray(fc1_w.T.reshape(HK, 128, I))
    fc1b = np.ascontiguousarray(fc1_b[:, None])
    fc2T = np.zeros((IA, T), np.float32)
    fc2T[:I] = fc2_w.reshape(T, I).T
    fc2b = np.asarray(fc2_b, np.float32).reshape(1, 1)

    shared = {
        "whhT": whhT, "wihT": wihT, "bhn": bhn, "fc1wT": fc1wT,
        "fc1b": fc1b, "fc2T": fc2T, "fc2b": fc2b,
        "onesv": np.ones((1, BL), np.float32),
    }
    in_maps = []
    for k in range(NCORES):
        m = dict(shared)
        m["frames"] = np.ascontiguousarray(frames_aug[:, :, k * BL:(k + 1) * BL])
        in_maps.append(m)
    return in_maps


def declare_io(nc):
    aps = {
        "frames": nc.dram_tensor("frames", [T, IA, BL], F32R, kind="ExternalInput").ap(),
        "whhT": nc.dram_tensor("whhT", [HK, 128, G3], F32R, kind="ExternalInput").ap(),
        "wihT": nc.dram_tensor("wihT", [IA, G3], F32R, kind="ExternalInput").ap(),
        "bhn": nc.dram_tensor("bhn", [1, H], F32R, kind="ExternalInput").ap(),
        "fc1wT": nc.dram_tensor("fc1wT", [HK, 128, I], F32R, kind="ExternalInput").ap(),
        "fc1b": nc.dram_tensor("fc1b", [I, 1], F32, kind="ExternalInput").ap(),
        "fc2T": nc.dram_tensor("fc2T", [IA, T], F32R, kind="ExternalInput").ap(),
        "fc2b": nc.dram_tensor("fc2b", [1, 1], F32, kind="ExternalInput").ap(),
        "onesv": nc.dram_tensor("onesv", [1, BL], F32R, kind="ExternalInput").ap(),
    }
    out_ap = nc.dram_tensor("out", [1, BL], F32, kind="ExternalOutput").ap()
    return aps, out_ap


def kernel(**inputs) -> np.ndarray:
    global LAST_RESULT
    in_maps = prep_inputs(inputs)

    nc = bacc.Bacc("TRN2", num_devices=NCORES, enable_asserts=False)
    aps, out_ap = declare_io(nc)
    build_gru(nc, aps, {"out": out_ap})
    nc.finalize()

    LAST_RESULT = run_bass_kernel_spmd(nc, in_maps, core_ids=list(range(NCORES)))

    out = np.empty((B, 1), np.float32)
    for k in range(NCORES):
        out[k * BL:(k + 1) * BL, 0] = LAST_RESULT.results[k]["out"][0]
    return out


# revision 20
# speedup vs baseline: 5311.5730x; 1.0117x over previous
"""Trainium2 Bass kernel: DiscriminatorRNN (GRU encode + autoregressive GRU decode).

Math (per reference):
  frames = concat(encoder_inputs, decoder_output) transposed to [T,B,I], T=74
  encode: h = GRUCell(frames[t], h)  for t in 0..73   (h0 = 0)
  decode: x0 = frames[0]; for d in 0..73: h = GRUCell(x_d, h); out_d = x_d + h@fc1_w.T + fc1_b; x_{d+1} = out_d
  logit[b] = sum_{d,i} out_d[b,i] * fc2_w[0, d*69+i] + fc2_b;  result = sigmoid(logit)  -> [B,1]

Distribution: pure data-parallel over batch, B=1024 -> 128 rows per core on 8 cores,
weights replicated, zero communication.

Per-core layout (one NeuronCore):
  - batch rows live on the matmul *stationary* operand (M=128), the GRU weights are
    the *moving* operand (float32r at full PE rate for free-dim >= 256).
  - gates g = x@Wih^T + h@Whh^T + biases accumulate in PSUM as [B=128, 3H] in six
    512-wide bursts (r0 r1 | z0 z1 | n0 n1), i_n and h_n kept in separate banks.
  - x is augmented with a constant 1.0 row; Wih^T gets a bias row, so biases ride the
    x matmul. h_n's b_hh bias comes from a K=1 ones-outer-product matmul.
  - the recurrent state is kept both as h [B,H] (for the elementwise update) and as
    h^T chunks [H,B] (the matmul stationary), refreshed each step by 8 PE transposes.
"""

import numpy as np

import concourse.bass as bass
from concourse import bacc
import concourse.mybir as mybir
import concourse.tile as tile
from concourse.bass_utils import run_bass_kernel_spmd
from concourse.masks import make_identity

B, SRC, TGT = 1024, 50, 25
I, H = 69, 1024
T = SRC + TGT - 1            # 74 frames
NCORES = 8
BL = B // NCORES             # 128 batch rows per core
IA = I + 1                   # 69 inputs + ones row (bias)
HK = H // 128                # 8 contraction chunks of h
G3 = 3 * H                   # 3072 gate columns (r|z|n)
NW = 512                     # burst width (one PSUM bank of fp32)

F32 = mybir.dt.float32
F32R = mybir.dt.float32r
BF16 = mybir.dt.bfloat16
AL = mybir.AluOpType
AF = mybir.ActivationFunctionType

LAST_RESULT = None           # BassKernelResults of the most recent run (for test.py)


def build_gru(nc, ins, outs, n_enc=T, n_dec=T):
    """Emit the full kernel into `nc`. `ins`/`outs` are dicts of DRAM APs."""
    # fp32r shares the fp32 byte layout; view f32-declared DRAM inputs as f32r
    # so the load DMAs are cast-free and verifier-visible as rounded producers.
    ins = {
        k: (v.bitcast(F32R) if k != "fc1b" and k != "fc2b" and v.dtype == F32 else v)
        for k, v in ins.items()
    }
    frames_d = ins["frames"]      # [T, IA, BL]
    whh_d = ins["whhT"]           # [HK, 128, G3]  whhT[c,k,n] = W_hh[n, c*128+k]
    wih_d = ins["wihT"]           # [IA, G3] rows 0:69 = W_ih^T, row 69 = bias row
    bhn_d = ins["bhn"]            # [1, H]   b_hh n-part
    fc1w_d = ins["fc1wT"]         # [HK, 128, I]
    fc1b_d = ins["fc1b"]          # [I, 1]
    fc2_d = ins["fc2T"]           # [IA, T]  rows 0:69 = per-step fc2 cols, row 69 = 0
    fc2b_d = ins["fc2b"]          # [1, 1]
    out_d = outs["out"]           # [1, BL]

    # burst name -> gate column offset; PSUM tag equals burst name
    cols = {"r0": 0, "r1": 512, "z0": 1024, "z1": 1536, "hn0": 2048, "hn1": 2560}
    border = ["r0", "hn0", "r1", "hn1", "z0", "z1"]

    with tile.TileContext(nc) as tc:
        with (
            tc.tile_pool(name="const", bufs=1) as const,
            tc.tile_pool(name="state", bufs=1) as state,
            tc.tile_pool(name="work", bufs=2) as work,
            tc.tile_pool(name="xin", bufs=4) as xin,
            tc.tile_pool(name="xout", bufs=3) as xop,
            tc.tile_pool(name="psum", bufs=1, space="PSUM") as psum,
        ):
            # ---- resident weights / constants ----
            whh_sb = const.tile([128, HK, G3], F32R)
            for c in range(HK):
                nc.sync.dma_start(whh_sb[:, c], whh_d[c])
            wih_sb = const.tile([IA, G3], F32R)
            nc.sync.dma_start(wih_sb, wih_d)
            bhn_sb = const.tile([1, H], F32R)
            nc.sync.dma_start(bhn_sb, bhn_d)
            fc1w_sb = const.tile([128, HK, I], F32R)
            for c in range(HK):
                nc.sync.dma_start(fc1w_sb[:, c], fc1w_d[c])
            fc1b_sb = const.tile([I, 1], F32)
            nc.sync.dma_start(fc1b_sb, fc1b_d)
            # bf16 copy of fc1 weights: N=128 fp32r matmuls run at 1/4 rate,
            # bf16 runs full rate; fc1's contribution tolerates bf16.
            fc1w_b = const.tile([128, HK, I], BF16)
            nc.vector.tensor_copy(out=fc1w_b, in_=fc1w_sb)
            fc2_sb = const.tile([IA, T], F32R)
            nc.sync.dma_start(fc2_sb, fc2_d)
            fc2b_sb = const.tile([1, 1], F32)
            nc.sync.dma_start(fc2b_sb, fc2b_d)
            ones_sb = const.tile([1, BL], F32R)
            nc.sync.dma_start(ones_sb, ins["onesv"])
            ident_g = const.tile([128, 128], F32)
            make_identity(nc, ident_g)
            # transposes depending directly on the gpsimd-built identity would
            # carry waits on 3 distinct semaphores (> the 2-slot LDW limit);
            # route it through DVE so its dep folds into the DVE semaphore.
            ident = const.tile([128, 128], F32)
            nc.vector.tensor_copy(out=ident, in_=ident_g)

            # ---- recurrent state ----
            h_zero = state.tile([128, H], F32)
            nc.vector.memset(h_zero, 0.0)
            h_buf = state.tile([128, H], F32)   # [B, H]
            nc.vector.tensor_copy(out=h_buf, in_=h_zero)
            # DVE memset can't encode f32r; tensor_copy with an f32r output is
            # the sanctioned rounding producer, so zero hT via a copy instead.
            hT = state.tile([128, H], F32R)      # chunk c at [:, c*128:(c+1)*128]
            nc.vector.tensor_copy(out=hT, in_=h_zero)
            hT_b = state.tile([128, H], BF16)    # bf16 twin, feeds fc1 (decode)
            nc.vector.tensor_copy(out=hT_b, in_=h_zero)
            acc = state.tile([1, BL], F32)      # fc2 logit accumulator

            def gru_step(x_sb, dec_idx):
                """One GRUCell step. x_sb: [IA, BL] sbuf tile (row 69 == 1.0).
                dec_idx: None for encode, else decode step index.
                Returns the new xout tile for decode steps."""
                g = {}
                for k in border:
                    g[k] = psum.tile([128, NW], F32, tag=k, name=f"g_{k}")
                gi = {
                    0: psum.tile([128, NW], F32, tag="pin0", name="gi_0"),
                    1: psum.tile([128, NW], F32, tag="pin1", name="gi_1"),
                }

                # On encode steps x arrives early by DMA: emit its matmuls first
                # (they prefetch into the previous step's tail). On decode steps
                # x is the previous step's fc1 output and lands *after* the
                # copybacks, so its matmuls go after the h-bursts instead and the
                # h-matmuls open the PSUM accumulation groups.
                late_x = dec_idx is not None

                def x_mm(k):
                    c0 = cols[k]
                    if k.startswith("hn"):
                        nc.tensor.matmul(
                            gi[int(k[2])], x_sb, wih_sb[:, c0:c0 + NW],
                            start=True, stop=True)
                    else:
                        nc.tensor.matmul(
                            g[k], x_sb, wih_sb[:, c0:c0 + NW],
                            start=not late_x, stop=late_x)

                for k in border:
                    c0 = cols[k]
                    if k.startswith("hn"):
                        # bias opener for the h_n banks (x-independent)
                        nc.tensor.matmul(
                            g[k], ones_sb, bhn_sb[:, c0 - 2 * H:c0 - 2 * H + NW],
                            start=True, stop=False)
                    if not late_x:
                        x_mm(k)

                # h-phase bursts + interleaved elementwise
                def hburst(k):
                    c0 = cols[k]
                    for c in range(HK):
                        nc.tensor.matmul(
                            g[k], hT[:, c * 128:(c + 1) * 128],
                            whh_sb[:, c, c0:c0 + NW],
                            start=(late_x and c == 0 and not k.startswith("hn")),
                            stop=(c == HK - 1 and (not late_x or k.startswith("hn"))))
                    if late_x:
                        x_mm(k)

                r_sb, n_sb, hmn_sb, z_sb = {}, {}, {}, {}

                def ew_n(j):
                    # after r{j} and hn{j} bursts: n_j = tanh(i_n + r*h_n); hmn = h - n
                    rj = work.tile([128, NW], F32, tag=f"r{j}_sb", name=f"r{j}_sb")
                    nc.scalar.activation(rj, g[f"r{j}"], AF.Sigmoid)
                    r_sb[j] = rj
                    rh = work.tile([128, NW], F32, tag=f"rh{j}", name=f"rh{j}")
                    nc.vector.tensor_tensor(rh, rj, g[f"hn{j}"], AL.mult)
                    npre = work.tile([128, NW], F32, tag=f"np{j}", name=f"np{j}")
                    nc.vector.tensor_tensor(npre, rh, gi[j], AL.add)
                    nj = work.tile([128, NW], F32, tag=f"n{j}_sb", name=f"n{j}_sb")
                    nc.scalar.activation(nj, npre, AF.Tanh)
                    n_sb[j] = nj
                    hm = work.tile([128, NW], F32, tag=f"hmn{j}", name=f"hmn{j}")
                    nc.vector.tensor_tensor(
                        hm, h_buf[:, j * NW:(j + 1) * NW], nj, AL.subtract)
                    hmn_sb[j] = hm

                def ew_z(j):
                    # after z{j} burst: h_new_j = n + z*(h-n), written into h_buf.
                    # Emitted per 256-wide quarter so transposes/copyback (and with
                    # them the next step's matmuls) unblock as early as possible.
                    zj = work.tile([128, NW], F32, tag=f"z{j}_sb", name=f"z{j}_sb")
                    zt = work.tile([128, NW], F32, tag=f"zt{j}", name=f"zt{j}")
                    for q in (0, 1):
                        s_ = slice(q * (NW // 2), (q + 1) * (NW // 2))
                        sg = slice(j * NW + q * (NW // 2), j * NW + (q + 1) * (NW // 2))
                        nc.scalar.activation(zj[:, s_], g[f"z{j}"][:, s_], AF.Sigmoid)
                        nc.vector.tensor_tensor(zt[:, s_], zj[:, s_], hmn_sb[j][:, s_], AL.mult)
                        nc.vector.tensor_tensor(
                            h_buf[:, sg], n_sb[j][:, s_], zt[:, s_], AL.add)
                    z_sb[j] = zj

                hburst("r0")
                hburst("hn0")
                ew_n(0)
                hburst("r1")
                hburst("hn1")
                ew_n(1)
                hburst("z0")
                ew_z(0)
                hburst("z1")
                ew_z(1)

                # refresh hT: 8 PE transposes (4 per half) + copyback
                tp = {
                    0: psum.tile([128, NW], F32, tag="z0", name="tp0"),
                    1: psum.tile([128, NW], F32, tag="z1", name="tp1"),
                }
                for j in (0, 1):
                    for i in range(4):
                        c = 4 * j + i
                        nc.tensor.transpose(
                            tp[j][:, i * 128:(i + 1) * 128],
                            h_buf[:, c * 128:(c + 1) * 128], ident)
                for j in (0, 1):
                    nc.any.tensor_copy(out=hT[:, j * NW:(j + 1) * NW], in_=tp[j])

                if dec_idx is None:
                    return None
                for j in (0, 1):
                    nc.any.tensor_copy(out=hT_b[:, j * NW:(j + 1) * NW], in_=tp[j])

                # decode extras: out_d = x_d + h@fc1_w.T + fc1_b ; logit += out_d @ fc2_d
                fp = psum.tile([I, BL], F32, tag="pin0", name="fc1_ps")
                for c in range(HK):
                    nc.tensor.matmul(
                        fp, fc1w_b[:, c], hT_b[:, c * 128:(c + 1) * 128],
                        start=(c == 0), stop=(c == HK - 1))
                xo = xop.tile([IA, BL], F32R, tag="xo", name="xo")
                # engines can't write partition-start 69 and DVE memset can't
                # encode f32r, so the constant row comes in by DMA.
                nc.sync.dma_start(xo[I:IA, :], ins["onesv"])
                nc.vector.scalar_tensor_tensor(
                    out=xo[0:I, :], in0=fp, scalar=fc1b_sb, in1=x_sb[0:I, :],
                    op0=AL.add, op1=AL.add)
                lp = psum.tile([1, BL], F32, tag="pin1", name="fc2_ps")
                nc.tensor.matmul(
                    lp, fc2_sb[:, dec_idx:dec_idx + 1], xo,
                    start=True, stop=True)
                if dec_idx == 0:
                    nc.vector.tensor_copy(out=acc, in_=lp)
                else:
                    nc.vector.tensor_tensor(acc, acc, lp, AL.add)
                return xo

            # ---- encode pass ----
            for t in range(n_enc):
                x_sb = xin.tile([IA, BL], F32R, tag="x", name="x")
                nc.sync.dma_start(x_sb, frames_d[t])
                gru_step(x_sb, None)

            # ---- decode pass ----
            x_sb = xin.tile([IA, BL], F32R, tag="x", name="x")
            nc.sync.dma_start(x_sb, frames_d[0])
            for d in range(n_dec):
                x_sb = gru_step(x_sb, d)

            # ---- final sigmoid + store ----
            res = state.tile([1, BL], F32)
            nc.scalar.activation(res, acc, AF.Sigmoid, bias=fc2b_sb[0:1, 0:1])
            nc.sync.dma_start(out_d, res)


def prep_inputs(inputs):
    """Host-side packing of the full-problem inputs into per-core DMA layouts."""
    enc = np.asarray(inputs["encoder_inputs"], np.float32)
    dec = np.asarray(inputs["decoder_output"], np.float32)
    w_ih = np.asarray(inputs["w_ih"], np.float32)
    w_hh = np.asarray(inputs["w_hh"], np.float32)
    b_ih = np.asarray(inputs["b_ih"], np.float32)
    b_hh = np.asarray(inputs["b_hh"], np.float32)
    fc1_w = np.asarray(inputs["fc1_w"], np.float32)
    fc1_b = np.asarray(inputs["fc1_b"], np.float32)
    fc2_w = np.asarray(inputs["fc2_w"], np.float32)
    fc2_b = np.asarray(inputs["fc2_b"], np.float32)

    all_frame = np.concatenate([enc, dec], axis=1)               # [B, T, I]
    framesT = all_frame.transpose(1, 2, 0)                       # [T, I, B]
    frames_aug = np.empty((T, IA, B), np.float32)
    frames_aug[:, :I] = framesT
    frames_aug[:, I] = 1.0

    whhT = np.ascontiguousarray(w_hh.T.reshape(HK, 128, G3))
    wihT = np.empty((IA, G3), np.float32)
    wihT[:I] = w_ih.T
    bsum = b_ih + b_hh
    wihT[I, :2 * H] = bsum[:2 * H]
    wihT[I, 2 * H:] = b_ih[2 * H:]
    bhn = np.ascontiguousarray(b_hh[2 * H:][None])               # [1, H]
    fc1wT = np.ascontiguousarray(fc1_w.T.reshape(HK, 128, I))
    fc1b = np.ascontiguousarray(fc1_b[:, None])
    fc2T = np.zeros((IA, T), np.float32)
    fc2T[:I] = fc2_w.reshape(T, I).T
    fc2b = np.asarray(fc2_b, np.float32).reshape(1, 1)

    shared = {
        "whhT": whhT, "wihT": wihT, "bhn": bhn, "fc1wT": fc1wT,
        "fc1b": fc1b, "fc2T": fc2T, "fc2b": fc2b,
        "onesv": np.ones((1, BL), np.float32),
    }
    in_maps = []
    for k in range(NCORES):
        m = dict(shared)
        m["frames"] = np.ascontiguousarray(frames_aug[:, :, k * BL:(k + 1) * BL])
        in_maps.append(m)
    return in_maps


def declare_io(nc):
    aps = {
        "frames": nc.dram_tensor("frames", [T, IA, BL], F32R, kind="ExternalInput").ap(),
        "whhT": nc.dram_tensor("whhT", [HK, 128, G3], F32R, kind="ExternalInput").ap(),
        "wihT": nc.dram_tensor("wihT", [IA, G3], F32R, kind="ExternalInput").ap(),
        "bhn": nc.dram_tensor("bhn", [1, H], F32R, kind="ExternalInput").ap(),
        "fc1wT": nc.dram_tensor("fc1wT", [HK, 128, I], F32R, kind="ExternalInput").ap(),
        "fc1b": nc.dram_tensor("fc1b", [I, 1], F32, kind="ExternalInput").ap(),
        "fc2T": nc.dram_tensor("fc2T", [IA, T], F32R, kind="ExternalInput").ap(),
        "fc2b": nc.dram_tensor("fc2b", [1, 1], F32, kind="ExternalInput").ap(),
        "onesv": nc.dram_tensor("onesv", [1, BL], F32R, kind="ExternalInput").ap(),
    }
    out_ap = nc.dram_tensor("out", [1, BL], F32, kind="ExternalOutput").ap()
    return aps, out_ap


def kernel(**inputs) -> np.ndarray:
    global LAST_RESULT
    in_maps = prep_inputs(inputs)

    nc = bacc.Bacc("TRN2", num_devices=NCORES, enable_asserts=False)
    aps, out_ap = declare_io(nc)
    build_gru(nc, aps, {"out": out_ap})
    nc.finalize()

    LAST_RESULT = run_bass_kernel_spmd(nc, in_maps, core_ids=list(range(NCORES)))

    out = np.empty((B, 1), np.float32)
    for k in range(NCORES):
        out[k * BL:(k + 1) * BL, 0] = LAST_RESULT.results[k]["out"][0]
    return out


# revision 21
# speedup vs baseline: 5836.6431x; 1.0989x over previous
"""Trainium2 Bass kernel: DiscriminatorRNN (GRU encode + autoregressive GRU decode).

Math (per reference):
  frames = concat(encoder_inputs, decoder_output) transposed to [T,B,I], T=74
  encode: h = GRUCell(frames[t], h)  for t in 0..73   (h0 = 0)
  decode: x0 = frames[0]; for d in 0..73: h = GRUCell(x_d, h); out_d = x_d + h@fc1_w.T + fc1_b; x_{d+1} = out_d
  logit[b] = sum_{d,i} out_d[b,i] * fc2_w[0, d*69+i] + fc2_b;  result = sigmoid(logit)  -> [B,1]

Distribution: pure data-parallel over batch, B=1024 -> 128 rows per core on 8 cores,
weights replicated, zero communication.

Per-core layout (one NeuronCore):
  - batch rows live on the matmul *stationary* operand (M=128), the GRU weights are
    the *moving* operand (float32r at full PE rate for free-dim >= 256).
  - gates g = x@Wih^T + h@Whh^T + biases accumulate in PSUM as [B=128, 3H] in six
    512-wide bursts (r0 r1 | z0 z1 | n0 n1), i_n and h_n kept in separate banks.
  - x is augmented with a constant 1.0 row; Wih^T gets a bias row, so biases ride the
    x matmul. h_n's b_hh bias comes from a K=1 ones-outer-product matmul.
  - the recurrent state is kept both as h [B,H] (for the elementwise update) and as
    h^T chunks [H,B] (the matmul stationary), refreshed each step by 8 PE transposes.
"""

import numpy as np

import concourse.bass as bass
from concourse import bacc
import concourse.mybir as mybir
import concourse.tile as tile
from concourse.bass_utils import run_bass_kernel_spmd
from concourse.masks import make_identity

B, SRC, TGT = 1024, 50, 25
I, H = 69, 1024
T = SRC + TGT - 1            # 74 frames
NCORES = 8
BL = B // NCORES             # 128 batch rows per core
IA = I + 1                   # 69 inputs + ones row (bias)
HK = H // 128                # 8 contraction chunks of h
G3 = 3 * H                   # 3072 gate columns (r|z|n)
NW = 512                     # burst width (one PSUM bank of fp32)

F32 = mybir.dt.float32
F32R = mybir.dt.float32r
BF16 = mybir.dt.bfloat16
AL = mybir.AluOpType
AF = mybir.ActivationFunctionType

LAST_RESULT = None           # BassKernelResults of the most recent run (for test.py)


def build_gru(nc, ins, outs, n_enc=T, n_dec=T):
    """Emit the full kernel into `nc`. `ins`/`outs` are dicts of DRAM APs."""
    # fp32r shares the fp32 byte layout; view f32-declared DRAM inputs as f32r
    # so the load DMAs are cast-free and verifier-visible as rounded producers.
    ins = {
        k: (v.bitcast(F32R) if k != "fc1b" and k != "fc2b" and v.dtype == F32 else v)
        for k, v in ins.items()
    }
    frames_d = ins["frames"]      # [T, IA, BL]
    whh_d = ins["whhT"]           # [HK, 128, G3]  whhT[c,k,n] = W_hh[n, c*128+k]
    wih_d = ins["wihT"]           # [IA, G3] rows 0:69 = W_ih^T, row 69 = bias row
    bhn_d = ins["bhn"]            # [1, H]   b_hh n-part
    fc1w_d = ins["fc1wT"]         # [HK, 128, I]
    fc1b_d = ins["fc1b"]          # [I, 1]
    fc2_d = ins["fc2T"]           # [IA, T]  rows 0:69 = per-step fc2 cols, row 69 = 0
    fc2b_d = ins["fc2b"]          # [1, 1]
    out_d = outs["out"]           # [1, BL]

    # burst name -> gate column offset; PSUM tag equals burst name
    cols = {"r0": 0, "r1": 512, "z0": 1024, "z1": 1536, "hn0": 2048, "hn1": 2560}
    border = ["r0", "hn0", "r1", "hn1", "z0", "z1"]

    with tile.TileContext(nc) as tc:
        with (
            tc.tile_pool(name="const", bufs=1) as const,
            tc.tile_pool(name="state", bufs=1) as state,
            tc.tile_pool(name="work", bufs=2) as work,
            tc.tile_pool(name="xin", bufs=4) as xin,
            tc.tile_pool(name="xout", bufs=3) as xop,
            tc.tile_pool(name="psum", bufs=1, space="PSUM") as psum,
        ):
            # ---- resident weights / constants ----
            whh_sb = const.tile([128, HK, G3], BF16)
            for c in range(HK):
                nc.sync.dma_start(whh_sb[:, c], whh_d[c])
            wih_sb = const.tile([IA, G3], F32R)
            nc.sync.dma_start(wih_sb, wih_d)
            bhn_sb = const.tile([1, H], F32R)
            nc.sync.dma_start(bhn_sb, bhn_d)
            fc1w_sb = const.tile([128, HK, I], F32R)
            for c in range(HK):
                nc.sync.dma_start(fc1w_sb[:, c], fc1w_d[c])
            fc1b_sb = const.tile([I, 1], F32)
            nc.sync.dma_start(fc1b_sb, fc1b_d)
            # bf16 copy of fc1 weights: N=128 fp32r matmuls run at 1/4 rate,
            # bf16 runs full rate; fc1's contribution tolerates bf16.
            fc1w_b = const.tile([128, HK, I], BF16)
            nc.vector.tensor_copy(out=fc1w_b, in_=fc1w_sb)
            fc2_sb = const.tile([IA, T], F32R)
            nc.sync.dma_start(fc2_sb, fc2_d)
            fc2b_sb = const.tile([1, 1], F32)
            nc.sync.dma_start(fc2b_sb, fc2b_d)
            ones_sb = const.tile([1, BL], F32R)
            nc.sync.dma_start(ones_sb, ins["onesv"])
            ident_g = const.tile([128, 128], F32)
            make_identity(nc, ident_g)
            # transposes depending directly on the gpsimd-built identity would
            # carry waits on 3 distinct semaphores (> the 2-slot LDW limit);
            # route it through DVE so its dep folds into the DVE semaphore.
            ident = const.tile([128, 128], BF16)
            nc.vector.tensor_copy(out=ident, in_=ident_g)

            # ---- recurrent state ----
            h_zero = state.tile([128, H], F32)
            nc.vector.memset(h_zero, 0.0)
            h_buf = state.tile([128, H], BF16)  # [B, H]
            nc.vector.tensor_copy(out=h_buf, in_=h_zero)
            # DVE memset can't encode f32r; tensor_copy with an f32r output is
            # the sanctioned rounding producer, so zero hT via a copy instead.
            hT = state.tile([128, H], BF16)      # chunk c at [:, c*128:(c+1)*128]
            nc.vector.tensor_copy(out=hT, in_=h_zero)
            acc = state.tile([1, BL], F32)      # fc2 logit accumulator

            def gru_step(x_sb, dec_idx):
                """One GRUCell step. x_sb: [IA, BL] sbuf tile (row 69 == 1.0).
                dec_idx: None for encode, else decode step index.
                Returns the new xout tile for decode steps."""
                g = {}
                for k in border:
                    g[k] = psum.tile([128, NW], F32, tag=k, name=f"g_{k}")
                gi = {
                    0: psum.tile([128, NW], F32, tag="pin0", name="gi_0"),
                    1: psum.tile([128, NW], F32, tag="pin1", name="gi_1"),
                }

                # On encode steps x arrives early by DMA: emit its matmuls first
                # (they prefetch into the previous step's tail). On decode steps
                # x is the previous step's fc1 output and lands *after* the
                # copybacks, so its matmuls go after the h-bursts instead and the
                # h-matmuls open the PSUM accumulation groups.
                late_x = dec_idx is not None

                def x_mm(k):
                    c0 = cols[k]
                    if k.startswith("hn"):
                        nc.tensor.matmul(
                            gi[int(k[2])], x_sb, wih_sb[:, c0:c0 + NW],
                            start=True, stop=True)
                    else:
                        nc.tensor.matmul(
                            g[k], x_sb, wih_sb[:, c0:c0 + NW],
                            start=not late_x, stop=late_x)

                for k in border:
                    c0 = cols[k]
                    if k.startswith("hn"):
                        # bias opener for the h_n banks (x-independent)
                        nc.tensor.matmul(
                            g[k], ones_sb, bhn_sb[:, c0 - 2 * H:c0 - 2 * H + NW],
                            start=True, stop=False)
                    if not late_x:
                        x_mm(k)

                # h-phase bursts + interleaved elementwise
                def hburst(k):
                    c0 = cols[k]
                    for c in range(HK):
                        nc.tensor.matmul(
                            g[k], hT[:, c * 128:(c + 1) * 128],
                            whh_sb[:, c, c0:c0 + NW],
                            start=(late_x and c == 0 and not k.startswith("hn")),
                            stop=(c == HK - 1 and (not late_x or k.startswith("hn"))))
                    if late_x:
                        x_mm(k)

                r_sb, n_sb, hmn_sb, z_sb = {}, {}, {}, {}

                def ew_n(j):
                    # after r{j} and hn{j} bursts: n_j = tanh(i_n + r*h_n); hmn = h - n
                    rj = work.tile([128, NW], F32, tag=f"r{j}_sb", name=f"r{j}_sb")
                    nc.scalar.activation(rj, g[f"r{j}"], AF.Sigmoid)
                    r_sb[j] = rj
                    rh = work.tile([128, NW], F32, tag=f"rh{j}", name=f"rh{j}")
                    nc.vector.tensor_tensor(rh, rj, g[f"hn{j}"], AL.mult)
                    npre = work.tile([128, NW], F32, tag=f"np{j}", name=f"np{j}")
                    nc.vector.tensor_tensor(npre, rh, gi[j], AL.add)
                    nj = work.tile([128, NW], BF16, tag=f"n{j}_sb", name=f"n{j}_sb")
                    nc.scalar.activation(nj, npre, AF.Tanh)
                    n_sb[j] = nj
                    hm = work.tile([128, NW], BF16, tag=f"hmn{j}", name=f"hmn{j}")
                    nc.vector.tensor_tensor(
                        hm, h_buf[:, j * NW:(j + 1) * NW], nj, AL.subtract)
                    hmn_sb[j] = hm

                def ew_z(j):
                    # after z{j} burst: h_new_j = n + z*(h-n), written into h_buf.
                    # Emitted per 256-wide quarter so transposes/copyback (and with
                    # them the next step's matmuls) unblock as early as possible.
                    zj = work.tile([128, NW], BF16, tag=f"z{j}_sb", name=f"z{j}_sb")
                    zt = work.tile([128, NW], BF16, tag=f"zt{j}", name=f"zt{j}")
                    for q in (0, 1):
                        s_ = slice(q * (NW // 2), (q + 1) * (NW // 2))
                        sg = slice(j * NW + q * (NW // 2), j * NW + (q + 1) * (NW // 2))
                        nc.scalar.activation(zj[:, s_], g[f"z{j}"][:, s_], AF.Sigmoid)
                        nc.vector.tensor_tensor(zt[:, s_], zj[:, s_], hmn_sb[j][:, s_], AL.mult)
                        nc.vector.tensor_tensor(
                            h_buf[:, sg], n_sb[j][:, s_], zt[:, s_], AL.add)
                    z_sb[j] = zj

                hburst("r0")
                hburst("hn0")
                ew_n(0)
                hburst("r1")
                hburst("hn1")
                ew_n(1)
                hburst("z0")
                ew_z(0)
                hburst("z1")
                ew_z(1)

                # refresh hT: 8 PE transposes (4 per half) + copyback
                tp = {
                    0: psum.tile([128, NW], BF16, tag="z0", name="tp0"),
                    1: psum.tile([128, NW], BF16, tag="z1", name="tp1"),
                }
                for j in (0, 1):
                    for i in range(4):
                        c = 4 * j + i
                        nc.tensor.transpose(
                            tp[j][:, i * 128:(i + 1) * 128],
                            h_buf[:, c * 128:(c + 1) * 128], ident)
                for j in (0, 1):
                    nc.any.tensor_copy(out=hT[:, j * NW:(j + 1) * NW], in_=tp[j])

                if dec_idx is None:
                    return None

                # decode extras: out_d = x_d + h@fc1_w.T + fc1_b ; logit += out_d @ fc2_d
                fp = psum.tile([I, BL], F32, tag="pin0", name="fc1_ps")
                for c in range(HK):
                    nc.tensor.matmul(
                        fp, fc1w_b[:, c], hT[:, c * 128:(c + 1) * 128],
                        start=(c == 0), stop=(c == HK - 1))
                xo = xop.tile([IA, BL], F32R, tag="xo", name="xo")
                # engines can't write partition-start 69 and DVE memset can't
                # encode f32r, so the constant row comes in by DMA.
                nc.sync.dma_start(xo[I:IA, :], ins["onesv"])
                nc.vector.scalar_tensor_tensor(
                    out=xo[0:I, :], in0=fp, scalar=fc1b_sb, in1=x_sb[0:I, :],
                    op0=AL.add, op1=AL.add)
                lp = psum.tile([1, BL], F32, tag="pin1", name="fc2_ps")
                nc.tensor.matmul(
                    lp, fc2_sb[:, dec_idx:dec_idx + 1], xo,
                    start=True, stop=True)
                if dec_idx == 0:
                    nc.vector.tensor_copy(out=acc, in_=lp)
                else:
                    nc.vector.tensor_tensor(acc, acc, lp, AL.add)
                return xo

            # ---- encode pass ----
            for t in range(n_enc):
                x_sb = xin.tile([IA, BL], F32R, tag="x", name="x")
                nc.sync.dma_start(x_sb, frames_d[t])
                gru_step(x_sb, None)

            # ---- decode pass ----
            x_sb = xin.tile([IA, BL], F32R, tag="x", name="x")
            nc.sync.dma_start(x_sb, frames_d[0])
            for d in range(n_dec):
                x_sb = gru_step(x_sb, d)

            # ---- final sigmoid + store ----
            res = state.tile([1, BL], F32)
            nc.scalar.activation(res, acc, AF.Sigmoid, bias=fc2b_sb[0:1, 0:1])
            nc.sync.dma_start(out_d, res)


def prep_inputs(inputs):
    """Host-side packing of the full-problem inputs into per-core DMA layouts."""
    enc = np.asarray(inputs["encoder_inputs"], np.float32)
    dec = np.asarray(inputs["decoder_output"], np.float32)
    w_ih = np.asarray(inputs["w_ih"], np.float32)
    w_hh = np.asarray(inputs["w_hh"], np.float32)
    b_ih = np.asarray(inputs["b_ih"], np.float32)
    b_hh = np.asarray(inputs["b_hh"], np.float32)
    fc1_w = np.asarray(inputs["fc1_w"], np.float32)
    fc1_b = np.asarray(inputs["fc1_b"], np.float32)
    fc2_w = np.asarray(inputs["fc2_w"], np.float32)
    fc2_b = np.asarray(inputs["fc2_b"], np.float32)

    all_frame = np.concatenate([enc, dec], axis=1)               # [B, T, I]
    framesT = all_frame.transpose(1, 2, 0)                       # [T, I, B]
    frames_aug = np.empty((T, IA, B), np.float32)
    frames_aug[:, :I] = framesT
    frames_aug[:, I] = 1.0

    import ml_dtypes
    whhT = np.ascontiguousarray(w_hh.T.reshape(HK, 128, G3)).astype(ml_dtypes.bfloat16)
    wihT = np.empty((IA, G3), np.float32)
    wihT[:I] = w_ih.T
    bsum = b_ih + b_hh
    wihT[I, :2 * H] = bsum[:2 * H]
    wihT[I, 2 * H:] = b_ih[2 * H:]
    bhn = np.ascontiguousarray(b_hh[2 * H:][None])               # [1, H]
    fc1wT = np.ascontiguousarray(fc1_w.T.reshape(HK, 128, I))
    fc1b = np.ascontiguousarray(fc1_b[:, None])
    fc2T = np.zeros((IA, T), np.float32)
    fc2T[:I] = fc2_w.reshape(T, I).T
    fc2b = np.asarray(fc2_b, np.float32).reshape(1, 1)

    shared = {
        "whhT": whhT, "wihT": wihT, "bhn": bhn, "fc1wT": fc1wT,
        "fc1b": fc1b, "fc2T": fc2T, "fc2b": fc2b,
        "onesv": np.ones((1, BL), np.float32),
    }
    in_maps = []
    for k in range(NCORES):
        m = dict(shared)
        m["frames"] = np.ascontiguousarray(frames_aug[:, :, k * BL:(k + 1) * BL])
        in_maps.append(m)
    return in_maps


def declare_io(nc):
    aps = {
        "frames": nc.dram_tensor("frames", [T, IA, BL], F32R, kind="ExternalInput").ap(),
        "whhT": nc.dram_tensor("whhT", [HK, 128, G3], BF16, kind="ExternalInput").ap(),
        "wihT": nc.dram_tensor("wihT", [IA, G3], F32R, kind="ExternalInput").ap(),
        "bhn": nc.dram_tensor("bhn", [1, H], F32R, kind="ExternalInput").ap(),
        "fc1wT": nc.dram_tensor("fc1wT", [HK, 128, I], F32R, kind="ExternalInput").ap(),
        "fc1b": nc.dram_tensor("fc1b", [I, 1], F32, kind="ExternalInput").ap(),
        "fc2T": nc.dram_tensor("fc2T", [IA, T], F32R, kind="ExternalInput").ap(),
        "fc2b": nc.dram_tensor("fc2b", [1, 1], F32, kind="ExternalInput").ap(),
        "onesv": nc.dram_tensor("onesv", [1, BL], F32R, kind="ExternalInput").ap(),
    }
    out_ap = nc.dram_tensor("out", [1, BL], F32, kind="ExternalOutput").ap()
    return aps, out_ap


def kernel(**inputs) -> np.ndarray:
    global LAST_RESULT
    in_maps = prep_inputs(inputs)

    nc = bacc.Bacc("TRN2", num_devices=NCORES, enable_asserts=False)
    aps, out_ap = declare_io(nc)
    build_gru(nc, aps, {"out": out_ap})
    nc.finalize()

    LAST_RESULT = run_bass_kernel_spmd(nc, in_maps, core_ids=list(range(NCORES)))

    out = np.empty((B, 1), np.float32)
    for k in range(NCORES):
        out[k * BL:(k + 1) * BL, 0] = LAST_RESULT.results[k]["out"][0]
    return out


# revision 24
# speedup vs baseline: 6159.7497x; 1.0554x over previous
"""Trainium2 Bass kernel: DiscriminatorRNN (GRU encode + autoregressive GRU decode).

Math (per reference):
  frames = concat(encoder_inputs, decoder_output) transposed to [T,B,I], T=74
  encode: h = GRUCell(frames[t], h)  for t in 0..73   (h0 = 0)
  decode: x0 = frames[0]; for d in 0..73: h = GRUCell(x_d, h); out_d = x_d + h@fc1_w.T + fc1_b; x_{d+1} = out_d
  logit[b] = sum_{d,i} out_d[b,i] * fc2_w[0, d*69+i] + fc2_b;  result = sigmoid(logit)  -> [B,1]

Distribution: pure data-parallel over batch, B=1024 -> 128 rows per core on 8 cores,
weights replicated, zero communication.

Per-core layout (one NeuronCore):
  - batch rows live on the matmul *stationary* operand (M=128), the GRU weights are
    the *moving* operand (float32r at full PE rate for free-dim >= 256).
  - gates g = x@Wih^T + h@Whh^T + biases accumulate in PSUM as [B=128, 3H] in six
    512-wide bursts (r0 r1 | z0 z1 | n0 n1), i_n and h_n kept in separate banks.
  - x is augmented with a constant 1.0 row; Wih^T gets a bias row, so biases ride the
    x matmul. h_n's b_hh bias comes from a K=1 ones-outer-product matmul.
  - the recurrent state is kept both as h [B,H] (for the elementwise update) and as
    h^T chunks [H,B] (the matmul stationary), refreshed each step by 8 PE transposes.
"""

import numpy as np

import concourse.bass as bass
from concourse import bacc
import concourse.mybir as mybir
import concourse.tile as tile
from concourse.bass_utils import run_bass_kernel_spmd
from concourse.masks import make_identity

B, SRC, TGT = 1024, 50, 25
I, H = 69, 1024
T = SRC + TGT - 1            # 74 frames
NCORES = 8
BL = B // NCORES             # 128 batch rows per core
IA = I + 1                   # 69 inputs + ones row (bias)
HK = H // 128                # 8 contraction chunks of h
G3 = 3 * H                   # 3072 gate columns (r|z|n)
NW = 512                     # burst width (one PSUM bank of fp32)

F32 = mybir.dt.float32
F32R = mybir.dt.float32r
BF16 = mybir.dt.bfloat16
AL = mybir.AluOpType
AF = mybir.ActivationFunctionType

LAST_RESULT = None           # BassKernelResults of the most recent run (for test.py)


def build_gru(nc, ins, outs, n_enc=T, n_dec=T):
    """Emit the full kernel into `nc`. `ins`/`outs` are dicts of DRAM APs."""
    # fp32r shares the fp32 byte layout; view f32-declared DRAM inputs as f32r
    # so the load DMAs are cast-free and verifier-visible as rounded producers.
    ins = {
        k: (v.bitcast(F32R) if k != "fc1b" and k != "fc2b" and v.dtype == F32 else v)
        for k, v in ins.items()
    }
    frames_d = ins["frames"]      # [T, IA, BL]
    whh_d = ins["whhT"]           # [HK, 128, G3]  whhT[c,k,n] = W_hh[n, c*128+k]
    wih_d = ins["wihT"]           # [IA, G3] rows 0:69 = W_ih^T, row 69 = bias row
    bhn_d = ins["bhn"]            # [1, H]   b_hh n-part
    fc1w_d = ins["fc1wT"]         # [HK, 128, I]
    fc1b_d = ins["fc1b"]          # [I, 1]
    fc2_d = ins["fc2T"]           # [IA, T]  rows 0:69 = per-step fc2 cols, row 69 = 0
    fc2b_d = ins["fc2b"]          # [1, 1]
    out_d = outs["out"]           # [1, BL]

    # burst name -> gate column offset; PSUM tag equals burst name
    cols = {"r0": 0, "r1": 512, "z0": 1024, "z1": 1536, "hn0": 2048, "hn1": 2560}
    border = ["r0", "hn0", "r1", "hn1", "z0", "z1"]

    with tile.TileContext(nc) as tc:
        with (
            tc.tile_pool(name="const", bufs=1) as const,
            tc.tile_pool(name="state", bufs=1) as state,
            tc.tile_pool(name="work", bufs=2) as work,
            tc.tile_pool(name="xin", bufs=4) as xin,
            tc.tile_pool(name="xout", bufs=3) as xop,
            tc.tile_pool(name="psum", bufs=1, space="PSUM") as psum,
        ):
            # ---- resident weights / constants ----
            whh_sb = const.tile([128, HK, G3], BF16)
            for c in range(HK):
                nc.sync.dma_start(whh_sb[:, c], whh_d[c])
            wih_sb = const.tile([IA, G3], BF16)
            nc.sync.dma_start(wih_sb, wih_d)
            bhn_sb = const.tile([1, H], BF16)
            nc.sync.dma_start(bhn_sb, bhn_d)
            fc1w_sb = const.tile([128, HK, I], F32R)
            for c in range(HK):
                nc.sync.dma_start(fc1w_sb[:, c], fc1w_d[c])
            fc1b_sb = const.tile([I, 1], F32)
            nc.sync.dma_start(fc1b_sb, fc1b_d)
            # bf16 copy of fc1 weights: N=128 fp32r matmuls run at 1/4 rate,
            # bf16 runs full rate; fc1's contribution tolerates bf16.
            fc1w_b = const.tile([128, HK, I], BF16)
            nc.vector.tensor_copy(out=fc1w_b, in_=fc1w_sb)
            fc2_sb = const.tile([IA, T], F32R)
            nc.sync.dma_start(fc2_sb, fc2_d)
            fc2b_sb = const.tile([1, 1], F32)
            nc.sync.dma_start(fc2b_sb, fc2b_d)
            ones_sb = const.tile([1, BL], BF16)
            nc.sync.dma_start(ones_sb, ins["onesv"])
            ident_g = const.tile([128, 128], F32)
            make_identity(nc, ident_g)
            # transposes depending directly on the gpsimd-built identity would
            # carry waits on 3 distinct semaphores (> the 2-slot LDW limit);
            # route it through DVE so its dep folds into the DVE semaphore.
            ident = const.tile([128, 128], BF16)
            nc.vector.tensor_copy(out=ident, in_=ident_g)

            # ---- recurrent state ----
            h_zero = state.tile([128, H], F32)
            nc.vector.memset(h_zero, 0.0)
            h_buf = state.tile([128, H], BF16)  # [B, H]
            nc.vector.tensor_copy(out=h_buf, in_=h_zero)
            # DVE memset can't encode f32r; tensor_copy with an f32r output is
            # the sanctioned rounding producer, so zero hT via a copy instead.
            hT = state.tile([128, H], BF16)      # chunk c at [:, c*128:(c+1)*128]
            nc.vector.tensor_copy(out=hT, in_=h_zero)
            acc = state.tile([1, BL], F32)      # fc2 logit accumulator

            def gru_step(x_sb, dec_idx, prev=None):
                """One GRUCell step. x_sb: [IA, BL] sbuf tile (row 69 == 1.0).
                dec_idx: None for encode, else decode step index.
                Returns the new xout tile for decode steps."""
                g = {}
                for k in border:
                    g[k] = psum.tile([128, NW], F32, tag=k, name=f"g_{k}")
                gi = {
                    0: psum.tile([128, NW], F32, tag="pin0", name="gi_0"),
                    1: psum.tile([128, NW], F32, tag="pin1", name="gi_1"),
                }

                # On encode steps x arrives early by DMA: emit its matmuls first
                # (they prefetch into the previous step's tail). On decode steps
                # x is the previous step's fc1 output and lands *after* the
                # copybacks, so its matmuls go after the h-bursts instead and the
                # h-matmuls open the PSUM accumulation groups.
                late_x = dec_idx is not None

                def x_mm(k):
                    c0 = cols[k]
                    if k.startswith("hn"):
                        nc.tensor.matmul(
                            gi[int(k[2])], x_sb, wih_sb[:, c0:c0 + NW],
                            start=True, stop=True)
                    else:
                        nc.tensor.matmul(
                            g[k], x_sb, wih_sb[:, c0:c0 + NW],
                            start=not late_x, stop=late_x)

                for k in border:
                    c0 = cols[k]
                    if k.startswith("hn"):
                        # bias opener for the h_n banks (x-independent)
                        nc.tensor.matmul(
                            g[k], ones_sb, bhn_sb[:, c0 - 2 * H:c0 - 2 * H + NW],
                            start=True, stop=False)
                    if not late_x:
                        x_mm(k)

                # h-phase bursts + interleaved elementwise
                def hburst(k):
                    c0 = cols[k]
                    for c in range(HK):
                        nc.tensor.matmul(
                            g[k], hT[:, c * 128:(c + 1) * 128],
                            whh_sb[:, c, c0:c0 + NW],
                            start=(late_x and c == 0 and not k.startswith("hn")),
                            stop=(c == HK - 1 and (not late_x or k.startswith("hn"))))
                    if late_x:
                        x_mm(k)

                r_sb, n_sb, hmn_sb, z_sb = {}, {}, {}, {}

                def ew_n(j):
                    # after r{j} and hn{j} bursts: n_j = tanh(i_n + r*h_n); hmn = h - n
                    rj = work.tile([128, NW], F32, tag=f"r{j}_sb", name=f"r{j}_sb")
                    nc.scalar.activation(rj, g[f"r{j}"], AF.Sigmoid)
                    r_sb[j] = rj
                    rh = work.tile([128, NW], F32, tag=f"rh{j}", name=f"rh{j}")
                    nc.vector.tensor_tensor(rh, rj, g[f"hn{j}"], AL.mult)
                    npre = work.tile([128, NW], F32, tag=f"np{j}", name=f"np{j}")
                    nc.vector.tensor_tensor(npre, rh, gi[j], AL.add)
                    nj = work.tile([128, NW], BF16, tag=f"n{j}_sb", name=f"n{j}_sb")
                    nc.scalar.activation(nj, npre, AF.Tanh)
                    n_sb[j] = nj
                    hm = work.tile([128, NW], BF16, tag=f"hmn{j}", name=f"hmn{j}")
                    nc.vector.tensor_tensor(
                        hm, h_buf[:, j * NW:(j + 1) * NW], nj, AL.subtract)
                    hmn_sb[j] = hm

                def ew_z(j):
                    # after z{j} burst: h_new_j = n + z*(h-n), written into h_buf.
                    # Emitted per 256-wide quarter so transposes/copyback (and with
                    # them the next step's matmuls) unblock as early as possible.
                    zj = work.tile([128, NW], BF16, tag=f"z{j}_sb", name=f"z{j}_sb")
                    zt = work.tile([128, NW], BF16, tag=f"zt{j}", name=f"zt{j}")
                    for q in (0, 1):
                        s_ = slice(q * (NW // 2), (q + 1) * (NW // 2))
                        sg = slice(j * NW + q * (NW // 2), j * NW + (q + 1) * (NW // 2))
                        nc.scalar.activation(zj[:, s_], g[f"z{j}"][:, s_], AF.Sigmoid)
                        nc.vector.tensor_tensor(zt[:, s_], zj[:, s_], hmn_sb[j][:, s_], AL.mult)
                        nc.vector.tensor_tensor(
                            h_buf[:, sg], n_sb[j][:, s_], zt[:, s_], AL.add)
                    z_sb[j] = zj

                hburst("r0")
                hburst("hn0")
                ew_n(0)
                hburst("r1")
                hburst("hn1")
                ew_n(1)
                hburst("z0")
                ew_z(0)
                hburst("z1")
                ew_z(1)

                # refresh hT: 8 PE transposes (4 per half) + copyback
                tp = {
                    0: psum.tile([128, NW], BF16, tag="z0", name="tp0"),
                    1: psum.tile([128, NW], BF16, tag="z1", name="tp1"),
                }
                for j in (0, 1):
                    for i in range(4):
                        c = 4 * j + i
                        nc.tensor.transpose(
                            tp[j][:, i * 128:(i + 1) * 128],
                            h_buf[:, c * 128:(c + 1) * 128], ident)
                for j in (0, 1):
                    nc.any.tensor_copy(out=hT[:, j * NW:(j + 1) * NW], in_=tp[j])

                if dec_idx is None:
                    return None

                # decode extras: out_d = x_d + h@fc1_w.T + fc1_b ; logit += out_d @ fc2_d
                fp = psum.tile([I, BL], F32, tag="pin0", name="fc1_ps")
                for c in range(HK):
                    nc.tensor.matmul(
                        fp, fc1w_b[:, c], hT[:, c * 128:(c + 1) * 128],
                        start=(c == 0), stop=(c == HK - 1))
                xo = xop.tile([IA, BL], F32R, tag="xo", name="xo")
                # engines can't write partition-start 69 and DVE memset can't
                # encode f32r, so the constant row comes in by DMA.
                nc.sync.dma_start(xo[I:IA, :], ins["frame0"][I:IA, :])
                nc.vector.scalar_tensor_tensor(
                    out=xo[0:I, :], in0=fp, scalar=fc1b_sb, in1=prev[0:I, :],
                    op0=AL.add, op1=AL.add)
                lp = psum.tile([1, BL], F32, tag="pin1", name="fc2_ps")
                nc.tensor.matmul(
                    lp, fc2_sb[:, dec_idx:dec_idx + 1], xo,
                    start=True, stop=True)
                if dec_idx == 0:
                    nc.vector.tensor_copy(out=acc, in_=lp)
                else:
                    nc.vector.tensor_tensor(acc, acc, lp, AL.add)
                # bf16 twin feeds the next step's gate matmuls (keeps the PE in
                # one dtype mode); the f32r xo carries the precise out-chain.
                xo_b = xop.tile([IA, BL], BF16, tag="xob", name="xo_b")
                nc.any.tensor_copy(out=xo_b, in_=xo)
                return xo_b, xo

            # ---- encode pass ----
            for t in range(n_enc):
                x_sb = xin.tile([IA, BL], BF16, tag="x", name="x")
                nc.sync.dma_start(x_sb, frames_d[t])
                gru_step(x_sb, None)

            # ---- decode pass ----
            x_sb = xin.tile([IA, BL], BF16, tag="x", name="x")
            nc.sync.dma_start(x_sb, frames_d[0])
            prev = state.tile([IA, BL], F32R)
            nc.sync.dma_start(prev, ins["frame0"])
            for d in range(n_dec):
                x_sb, prev = gru_step(x_sb, d, prev)

            # ---- final sigmoid + store ----
            res = state.tile([1, BL], F32)
            nc.scalar.activation(res, acc, AF.Sigmoid, bias=fc2b_sb[0:1, 0:1])
            nc.sync.dma_start(out_d, res)


def prep_inputs(inputs):
    """Host-side packing of the full-problem inputs into per-core DMA layouts."""
    enc = np.asarray(inputs["encoder_inputs"], np.float32)
    dec = np.asarray(inputs["decoder_output"], np.float32)
    w_ih = np.asarray(inputs["w_ih"], np.float32)
    w_hh = np.asarray(inputs["w_hh"], np.float32)
    b_ih = np.asarray(inputs["b_ih"], np.float32)
    b_hh = np.asarray(inputs["b_hh"], np.float32)
    fc1_w = np.asarray(inputs["fc1_w"], np.float32)
    fc1_b = np.asarray(inputs["fc1_b"], np.float32)
    fc2_w = np.asarray(inputs["fc2_w"], np.float32)
    fc2_b = np.asarray(inputs["fc2_b"], np.float32)

    all_frame = np.concatenate([enc, dec], axis=1)               # [B, T, I]
    framesT = all_frame.transpose(1, 2, 0)                       # [T, I, B]
    frames_aug = np.empty((T, IA, B), np.float32)
    frames_aug[:, :I] = framesT
    frames_aug[:, I] = 1.0

    import ml_dtypes
    whhT = np.ascontiguousarray(w_hh.T.reshape(HK, 128, G3)).astype(ml_dtypes.bfloat16)
    wihT = np.empty((IA, G3), np.float32)
    wihT[:I] = w_ih.T
    bsum = b_ih + b_hh
    wihT[I, :2 * H] = bsum[:2 * H]
    wihT[I, 2 * H:] = b_ih[2 * H:]
    bhn = np.ascontiguousarray(b_hh[2 * H:][None])               # [1, H]
    fc1wT = np.ascontiguousarray(fc1_w.T.reshape(HK, 128, I))
    fc1b = np.ascontiguousarray(fc1_b[:, None])
    fc2T = np.zeros((IA, T), np.float32)
    fc2T[:I] = fc2_w.reshape(T, I).T
    fc2b = np.asarray(fc2_b, np.float32).reshape(1, 1)

    shared = {
        "whhT": whhT, "wihT": wihT.astype(ml_dtypes.bfloat16),
        "bhn": bhn.astype(ml_dtypes.bfloat16), "fc1wT": fc1wT,
        "fc1b": fc1b, "fc2T": fc2T, "fc2b": fc2b,
        "onesv": np.ones((1, BL), ml_dtypes.bfloat16),
    }
    in_maps = []
    for k in range(NCORES):
        m = dict(shared)
        fa = np.ascontiguousarray(frames_aug[:, :, k * BL:(k + 1) * BL])
        m["frames"] = fa.astype(ml_dtypes.bfloat16)
        m["frame0"] = fa[0]
        in_maps.append(m)
    return in_maps


def declare_io(nc):
    aps = {
        "frames": nc.dram_tensor("frames", [T, IA, BL], BF16, kind="ExternalInput").ap(),
        "whhT": nc.dram_tensor("whhT", [HK, 128, G3], BF16, kind="ExternalInput").ap(),
        "wihT": nc.dram_tensor("wihT", [IA, G3], BF16, kind="ExternalInput").ap(),
        "bhn": nc.dram_tensor("bhn", [1, H], BF16, kind="ExternalInput").ap(),
        "fc1wT": nc.dram_tensor("fc1wT", [HK, 128, I], F32R, kind="ExternalInput").ap(),
        "fc1b": nc.dram_tensor("fc1b", [I, 1], F32, kind="ExternalInput").ap(),
        "fc2T": nc.dram_tensor("fc2T", [IA, T], F32R, kind="ExternalInput").ap(),
        "fc2b": nc.dram_tensor("fc2b", [1, 1], F32, kind="ExternalInput").ap(),
        "onesv": nc.dram_tensor("onesv", [1, BL], BF16, kind="ExternalInput").ap(),
        "frame0": nc.dram_tensor("frame0", [IA, BL], F32R, kind="ExternalInput").ap(),
    }
    out_ap = nc.dram_tensor("out", [1, BL], F32, kind="ExternalOutput").ap()
    return aps, out_ap


def kernel(**inputs) -> np.ndarray:
    global LAST_RESULT
    in_maps = prep_inputs(inputs)

    nc = bacc.Bacc("TRN2", num_devices=NCORES, enable_asserts=False)
    aps, out_ap = declare_io(nc)
    build_gru(nc, aps, {"out": out_ap})
    nc.finalize()

    LAST_RESULT = run_bass_kernel_spmd(nc, in_maps, core_ids=list(range(NCORES)))

    out = np.empty((B, 1), np.float32)
    for k in range(NCORES):
        out[k * BL:(k + 1) * BL, 0] = LAST_RESULT.results[k]["out"][0]
    return out
